# revision 18
# baseline (speedup 1.0000x reference)
"""Multi-head attention block (pre-LN, residual) on 8 Trainium2 NeuronCores.

Sharding: (batch x head-group) grid. Core c handles batch b = c//2 and head
group g = c%2 (8 of 16 heads). Per core: LN(x_b) -> per-head QKV projections
-> softmax attention (no max-subtraction; scores are O(10)) -> out-projection
against the local 512-wide slice of Wo, + 0.5*(x+bo) residual. Host sums the
two partial outputs per batch (the pair all-reduce) and stacks batches.

v2 structure:
- LN apply moved to the Act engine (Identity activation with per-partition
  scale/bias), stats stay on DVE.
- xn -> xnT via DMA crossbar transposes (one per s-tile) instead of PE
  identity-matmul transposes + vector copies.
- PV matmuls flipped to [s, e] output orientation (free size 65 instead of
  512) with a ones-column appended to V so the softmax denominator falls out
  of the PE accumulation for free; normalization is folded into the
  PSUM->SBUF copy. Removes all denominator adds and broadcast-back matmuls.
- Attention heads are concatenated in [s, he] layout (hC) and DMA-crossbar
  transposed per s-tile into hT for the out-projection.
- Scores accumulate in bf16 PSUM so an exp instruction spans 2 t-tiles
  (2048 elements), halving Act-engine instruction overhead.

LayerNorm gamma/beta are folded into the QKV weights/biases on the host
(exact: projections are linear in xn). The 1/sqrt(E) score scale is folded
into Wq. Matmul operands are bf16 with fp32 PSUM accumulation for
projections/PV/out-proj; LN statistics and the residual path stay fp32.
"""

import numpy as np
import ml_dtypes

import concourse.bass as bass
import concourse.mybir as mybir
import concourse.tile as tile
from concourse import bacc
from concourse import bass_utils
from concourse.bass import ts

BF_NP = ml_dtypes.bfloat16

B, S, D = 4, 2048, 1024
H, E = 16, 64
LN_EPS = 1e-5
SCALE = 8.0  # sqrt(E) * TEMP

N_CORES = 8
HL = H // 2          # heads per core
NP_ = HL // 2        # 4 head pairs per core
KT = D // 128        # 8 contraction tiles for D
ST = S // 128        # 16 s-tiles of 128
NB = S // 512        # 4 s-blocks of 512
TT = S // 128        # 16 t-tiles of 128

F32 = mybir.dt.float32
BF = mybir.dt.bfloat16

# Score PSUM dtype: bf16 lets one exp instruction span TSPAN=2 t-tiles within
# the PSUM budget. Fallback (if bf16 matmul-out to PSUM misbehaves):
# F32S=True -> f32 scores, TSPAN=1.
F32S = True
TSPAN = 1 if F32S else 2
GT = TT // TSPAN     # groups per block
SDT = F32 if F32S else BF

_NC_CACHE = None


def _emit(nc, aps):
    x_ap = aps["x"]
    xr_ap = aps["xr"]
    wq_ap, wk_ap, wv_ap, wo_ap = aps["wq"], aps["wk"], aps["wv"], aps["wo"]
    bq_ap, bk_ap, bv_ap = aps["bq"], aps["bk"], aps["bv"]
    out_ap = aps["out"]

    tc = aps["tc"]
    import contextlib

    ctx = contextlib.ExitStack()
    with ctx:
        const = ctx.enter_context(tc.tile_pool(name="const", bufs=1))
        big = ctx.enter_context(tc.tile_pool(name="big", bufs=1))
        xin = ctx.enter_context(tc.tile_pool(name="xin", bufs=12))
        stat = ctx.enter_context(tc.tile_pool(name="stat", bufs=8))
        xnp = ctx.enter_context(tc.tile_pool(name="xnp", bufs=6))
        ptp = ctx.enter_context(tc.tile_pool(name="ptp", bufs=3))
        r8p = ctx.enter_context(tc.tile_pool(name="r8p", bufs=2))
        xrp = ctx.enter_context(tc.tile_pool(name="xrp", bufs=2))
        outp = ctx.enter_context(tc.tile_pool(name="outp", bufs=4))
        psA = ctx.enter_context(tc.tile_pool(name="psA", bufs=2, space="PSUM"))
        psS = ctx.enter_context(tc.tile_pool(name="psS", bufs=2, space="PSUM"))
        psB = ctx.enter_context(tc.tile_pool(name="psB", bufs=1, space="PSUM"))

        # ---- constants / weights resident in SBUF ----
        # wv/wk ride the fast HWDGE (sync) queue right after the first x
        # tiles: V(0) needs wv at ~7us, K(0,0) needs wk at ~10us. The SWDGE
        # (gpsimd) path costs ~1us Pool-engine time per DMA, which starved
        # the projections when all weights went through it.
        wq_sb = const.tile([128, KT, 512], BF, tag="wq")
        wk_sb = const.tile([128, KT, 512], BF, tag="wk")
        wv_sb = const.tile([128, KT, 512], BF, tag="wv")
        wo_sb = const.tile([128, 4, 1024], BF, tag="wo")
        bq_sb = const.tile([128, NP_], F32, tag="bq")
        bk_sb = const.tile([128, NP_], F32, tag="bk")
        bv_sb = const.tile([128, HL, E], F32, tag="bv")
        bv_bcast = bass.AP(
            tensor=bv_ap.tensor,
            offset=bv_ap.offset,
            ap=[[0, 128], [1, 512]],
        )
        # all weights ride the gpsimd SWDGE queue (Pool engine is otherwise
        # idle), most-urgent first: bv/wv for V(0) at ~9us, wk for K(p0),
        # wq/wo much later. Keeps both HWDGE queues free for x tiles and
        # transposes.
        nc.gpsimd.dma_start(out=bv_sb, in_=bv_bcast)
        nc.gpsimd.dma_start(out=bq_sb, in_=bq_ap)
        nc.gpsimd.dma_start(out=bk_sb, in_=bk_ap)
        for k in range(KT):
            nc.gpsimd.dma_start(out=wv_sb[:, k, :], in_=wv_ap[k])
        for k in range(KT):
            nc.gpsimd.dma_start(out=wk_sb[:, k, :], in_=wk_ap[k])
        for k in range(KT):
            nc.gpsimd.dma_start(out=wq_sb[:, k, :], in_=wq_ap[k])
        for k in range(4):
            nc.gpsimd.dma_start(out=wo_sb[:, k, :], in_=wo_ap[k])
        eps_t = const.tile([128, 1], F32, tag="eps")
        nc.vector.memset(eps_t, LN_EPS)

        # [d%128, s_tile, d//128, s%128] LN(x) transposed, per-tile contiguous
        xnT = big.tile([128, ST, KT, 128], BF, tag="xnT")
        qT = big.tile([128, NP_, S], BF, tag="qT")      # [(pairhead,e), s]
        kT_ = big.tile([128, NP_, S], BF, tag="kT")
        v_sb = big.tile([128, TT, HL, E + 1], BF, tag="v")  # [t, h, e | ones]
        hC = big.tile([128, ST, 512], BF, tag="hC")     # [s, (h,e)] attn out
        hT = big.tile([128, ST, 4, 128], BF, tag="hT")  # [(he)%128, s_tile, (he)//128, s%128]
        # ones column of V' (denominator accumulator feed)
        nc.gpsimd.memset(v_sb[:, :, :, E : E + 1], 1.0)

        # ---- work units ----
        # Phase-A PSUM drains are deferred to the next tile so no in-order
        # DVE instruction ever waits on the transpose->matmul chain (a DVE
        # stall there blocks the next tile's LN stats behind it).
        drains = []

        def flush_drains():
            while drains:
                drains.pop(0)()

        def emit_qk(kind, p, n, defer=False):
            w_sb, b_sb, dst = (
                (wq_sb, bq_sb, qT) if kind == "q" else (wk_sb, bk_sb, kT_)
            )
            ps = psA.tile([128, 512], F32, tag="ps", name=f"proj_{kind}_{p}_{n}")
            for k in range(KT):
                nc.tensor.matmul(
                    ps, lhsT=w_sb[:, k, ts(p, 128)],
                    rhs=xnT[:, 4 * n : 4 * n + 4, k, :],
                    start=(k == 0), stop=(k == KT - 1),
                )

            def drain():
                nc.vector.tensor_scalar_add(
                    out=dst[:, p, ts(n, 512)], in0=ps, scalar1=b_sb[:, p : p + 1]
                )

            if defer:
                drains.append(drain)
            else:
                drain()

        def emit_v(t):
            ps = psA.tile([128, HL, E], F32, tag="ps", name=f"proj_v_{t}")
            for k in range(KT):
                nc.tensor.matmul(
                    ps, lhsT=xnT[:, t, k, :], rhs=wv_sb[:, k, :],
                    start=(k == 0), stop=(k == KT - 1),
                )
            drains.append(
                lambda: nc.vector.tensor_add(
                    out=v_sb[:, t, :, 0:E], in0=ps, in1=bv_sb
                )
            )

        def emit_scores(p, n, g):
            s12 = psS.tile([128, TSPAN, 2, 512], SDT, tag="s12",
                           name=f"s12_{p}_{n}_{g}")
            for tt in range(TSPAN):
                t = TSPAN * g + tt
                nc.tensor.matmul(
                    s12[:, tt, 0, :], lhsT=kT_[0:64, p, ts(t, 128)],
                    rhs=qT[0:64, p, ts(n, 512)],
                    start=True, stop=True, tile_position=(0, 0),
                )
                nc.tensor.matmul(
                    s12[:, tt, 1, :], lhsT=kT_[64:128, p, ts(t, 128)],
                    rhs=qT[64:128, p, ts(n, 512)],
                    start=True, stop=True, tile_position=(64, 0),
                )
            return s12

        def emit_out_tile(i):
            xr_t = xrp.tile([128, D], F32, tag="xr", name=f"xr_{i}")
            nc.gpsimd.dma_start(out=xr_t, in_=xr_ap[ts(i, 128), :])
            for c in range(2):
                ps_o = psA.tile([128, 512], F32, tag="ps", name=f"pso_{i}_{c}")
                for k in range(4):
                    nc.tensor.matmul(
                        ps_o, lhsT=hT[:, i, k, :], rhs=wo_sb[:, k, ts(c, 512)],
                        start=(k == 0), stop=(k == 3),
                    )
                osb = outp.tile([128, 512], F32, tag="ob", name=f"ob_{i}_{c}")
                nc.vector.tensor_add(out=osb, in0=ps_o, in1=xr_t[:, ts(c, 512)])
                nc.gpsimd.dma_start(out=out_ap[ts(i, 128), ts(c, 512)], in_=osb)

        # ---- phase A: LN + transpose-DMA; V and pair-0 K/Q inline ----
        x_tiles = {}

        def issue_x(i):
            if i < ST and i not in x_tiles:
                x_tiles[i] = xin.tile([128, D], BF, tag="x", name=f"x_{i}")
                nc.sync.dma_start(out=x_tiles[i], in_=x_ap[ts(i, 128), :])

        for i in range(12):
            issue_x(i)
        for i in range(ST):
            x_t = x_tiles.pop(i)
            stats = stat.tile([128, 2, 6], F32, tag="st")
            for sg in range(2):
                nc.vector.bn_stats(out=stats[:, sg, :], in_=x_t[:, ts(sg, 512)])
            mv = stat.tile([128, 2], F32, tag="mv")
            nc.vector.bn_aggr(out=mv, in_=stats)
            std = stat.tile([128, 1], F32, tag="sd")
            nc.scalar.activation(
                out=std, in_=mv[:, 1:2],
                func=mybir.ActivationFunctionType.Sqrt, bias=eps_t,
            )
            istd = stat.tile([128, 1], F32, tag="is")
            nc.vector.reciprocal(out=istd, in_=std)
            xn_t = xnp.tile([128, D], BF, tag="xn")
            nc.vector.tensor_scalar(
                out=xn_t, in0=x_t,
                scalar1=mv[:, 0:1], scalar2=istd,
                op0=mybir.AluOpType.subtract, op1=mybir.AluOpType.mult,
            )
            nc.sync.dma_start_transpose(xnT[:, i], xn_t)
            issue_x(i + 12)
            flush_drains()
            if i >= 1:
                emit_v(i - 1)
            if i % 4 == 0 and i > 0:
                emit_qk("k", 0, i // 4 - 1, defer=True)
        emit_v(ST - 1)
        emit_qk("k", 0, NB - 1, defer=True)
        emit_qk("q", 0, 0, defer=True)
        flush_drains()

        # later pairs' projections stream into the Act-bound attention window
        work_queue = (
            [("q", 0, n) for n in range(1, NB)]
            + [(kind, p, n) for p in range(1, NP_)
               for kind in ("k", "q") for n in range(NB)]
        )

        def pop_work():
            if work_queue:
                emit_qk(*work_queue.pop(0))

        if TSPAN == 2:
            pop_slots = (2, 5)
            out_slots = (1, 3, 5, 7)
        else:
            pop_slots = (5, 11)
            out_slots = (2, 6, 10, 14)

        # ---- phase B: attention per (head-pair, s-block) ----
        blocks = [(p, n) for p in range(NP_) for n in range(NB)]
        s12_next = emit_scores(0, 0, 0)
        for bi, (p, n) in enumerate(blocks):
            psv = [
                psB.tile([128, 4, E + 1], F32, tag=f"pv{h}", name=f"pv_{p}_{n}_{h}")
                for h in range(2)
            ]
            for g in range(GT):
                s12 = s12_next
                pt = ptp.tile([128, TSPAN, 2, 512], BF, tag="pt",
                              name=f"pt_{p}_{n}_{g}")
                nc.scalar.activation(
                    out=pt, in_=s12, func=mybir.ActivationFunctionType.Exp
                )
                # prefetch next scores tile so PE stays busy during exp
                if g < GT - 1:
                    s12_next = emit_scores(p, n, g + 1)
                elif bi < len(blocks) - 1:
                    s12_next = emit_scores(*blocks[bi + 1], 0)
                if g in pop_slots:
                    pop_work()
                if p == NP_ - 1 and n >= 1 and g in out_slots:
                    emit_out_tile(4 * (n - 1) + out_slots.index(g))
                for tt in range(TSPAN):
                    t = TSPAN * g + tt
                    for h in range(2):
                        hg = 2 * p + h
                        for sc in range(4):
                            # one start/stop per PSUM bank: the sc==0 start
                            # zeroes the whole 2KB region for all 4 slots
                            nc.tensor.matmul(
                                psv[h][:, sc, :],
                                lhsT=pt[:, tt, h, ts(sc, 128)],
                                rhs=v_sb[:, t, hg, :],
                                start=(t == 0 and sc == 0),
                                stop=(t == TT - 1 and sc == 3),
                                skip_group_check=True,
                            )
            # block epilogue: normalize into hC (denominator is psv[..., E])
            r8 = r8p.tile([128, HL], F32, tag="r8", name=f"r8_{p}_{n}")
            for h in range(2):
                nc.vector.reciprocal(out=r8[:, 4 * h : 4 * h + 4], in_=psv[h][:, :, E])
            for j in range(HL):
                h, sc = j // 4, j % 4
                nc.vector.tensor_scalar_mul(
                    out=hC[:, n * 4 + sc, ts(2 * p + h, E)],
                    in0=psv[h][:, sc, 0:E],
                    scalar1=r8[:, j : j + 1],
                )
            if p == NP_ - 1:
                for sc in range(4):
                    i = n * 4 + sc
                    nc.sync.dma_start_transpose(hT[:, i], hC[:, i, :])
        # remaining out-projection tiles
        for i in range(4 * (NB - 1), ST):
            emit_out_tile(i)

        aps["_dbg"] = {
            "xnT": xnT.tensor.name, "qT": qT.tensor.name, "kT_": kT_.tensor.name,
            "v_sb": v_sb.tensor.name, "hC": hC.tensor.name, "hT": hT.tensor.name,
        }


def build():
    nc = bacc.Bacc("TRN2", target_bir_lowering=False, debug=False, num_devices=N_CORES)
    aps = {
        "x": nc.dram_tensor("x", [S, D], BF, kind="ExternalInput").ap(),
        "xr": nc.dram_tensor("xr", [S, D], F32, kind="ExternalInput").ap(),
        "wq": nc.dram_tensor("wq", [KT, 128, 512], BF, kind="ExternalInput").ap(),
        "wk": nc.dram_tensor("wk", [KT, 128, 512], BF, kind="ExternalInput").ap(),
        "wv": nc.dram_tensor("wv", [KT, 128, 512], BF, kind="ExternalInput").ap(),
        "wo": nc.dram_tensor("wo", [4, 128, 1024], BF, kind="ExternalInput").ap(),
        "bq": nc.dram_tensor("bq", [128, NP_], F32, kind="ExternalInput").ap(),
        "bk": nc.dram_tensor("bk", [128, NP_], F32, kind="ExternalInput").ap(),
        "bv": nc.dram_tensor("bv", [512], F32, kind="ExternalInput").ap(),
        "out": nc.dram_tensor("out", [S, D], F32, kind="ExternalOutput").ap(),
    }
    with tile.TileContext(nc) as tc:
        aps["tc"] = tc
        _emit(nc, aps)
    nc._dbg_names = aps.get("_dbg", {})
    nc.compile()
    return nc


def prep_core_inputs(x, Wq, bq, Wk, bk, Wv, bv, Wo, bo, ln_gamma, ln_beta):
    """Host-side sharding: returns list of 8 in_maps (numpy arrays)."""
    x = np.asarray(x, np.float32)
    Wq, bq = np.asarray(Wq, np.float32), np.asarray(bq, np.float32)
    Wk, bk = np.asarray(Wk, np.float32), np.asarray(bk, np.float32)
    Wv, bv = np.asarray(Wv, np.float32), np.asarray(bv, np.float32)
    Wo, bo = np.asarray(Wo, np.float32), np.asarray(bo, np.float32)
    gamma, beta = np.asarray(ln_gamma, np.float32), np.asarray(ln_beta, np.float32)

    Wq_eff = Wq * gamma[None, None, :] / SCALE
    bq_eff = (bq + Wq @ beta) / SCALE
    Wk_eff = Wk * gamma[None, None, :]
    bk_eff = bk + Wk @ beta
    Wv_eff = Wv * gamma[None, None, :]
    bv_eff = bv + Wv @ beta

    def wq_layout(w):  # [8, 64, 1024] -> [KT, 128, 512]
        # w[h, e, kt*128+dd] -> out[kt, dd, h*64+e]
        return np.ascontiguousarray(
            w.reshape(HL * E, KT, 128).transpose(1, 2, 0)
        ).astype(BF_NP)

    def b_layout(b):  # [8, 64] -> [128, 4]: out[(hh*64+e), p] = b[2p+hh, e]
        return np.ascontiguousarray(
            b.reshape(NP_, 2 * E).T
        ).astype(np.float32)

    in_maps = []
    for c in range(N_CORES):
        bidx, g = c // 2, c % 2
        hs = slice(g * HL, (g + 1) * HL)
        wo_loc = Wo[:, g * 512:(g + 1) * 512]  # [1024, 512]
        wo_dev = np.ascontiguousarray(
            wo_loc.T.reshape(4, 128, 1024)
        ).astype(BF_NP)
        in_maps.append({
            "x": x[bidx].astype(BF_NP),
            "xr": 0.5 * (x[bidx] + bo[None, :]),
            "wq": wq_layout(Wq_eff[hs]),
            "wk": wq_layout(Wk_eff[hs]),
            "wv": wq_layout(Wv_eff[hs]),
            "wo": wo_dev,
            "bq": b_layout(bq_eff[hs]),
            "bk": b_layout(bk_eff[hs]),
            "bv": bv_eff[hs].reshape(512).astype(np.float32),
            "out": np.zeros((S, D), np.float32),
        })
    return in_maps


def kernel(x, Wq, bq, Wk, bk, Wv, bv, Wo, bo, ln_gamma, ln_beta):
    global _NC_CACHE
    if _NC_CACHE is None:
        _NC_CACHE = build()
    nc = _NC_CACHE
    in_maps = prep_core_inputs(x, Wq, bq, Wk, bk, Wv, bv, Wo, bo, ln_gamma, ln_beta)
    for m in in_maps:
        m.pop("out")
    res = bass_utils.run_bass_kernel_spmd(nc, in_maps, core_ids=list(range(N_CORES)))
    out = np.empty((B, S, D), np.float32)
    for bidx in range(B):
        out[bidx] = res.results[2 * bidx]["out"] + res.results[2 * bidx + 1]["out"]
    return out


# revision 19
# speedup vs baseline: 1.0314x; 1.0314x over previous
"""Multi-head attention block (pre-LN, residual) on 8 Trainium2 NeuronCores.

Sharding: (batch x head-group) grid. Core c handles batch b = c//2 and head
group g = c%2 (8 of 16 heads). Per core: LN(x_b) -> per-head QKV projections
-> softmax attention (no max-subtraction; scores are O(10)) -> out-projection
against the local 512-wide slice of Wo, + 0.5*(x+bo) residual. Host sums the
two partial outputs per batch (the pair all-reduce) and stacks batches.

v2 structure:
- LN apply moved to the Act engine (Identity activation with per-partition
  scale/bias), stats stay on DVE.
- xn -> xnT via DMA crossbar transposes (one per s-tile) instead of PE
  identity-matmul transposes + vector copies.
- PV matmuls flipped to [s, e] output orientation (free size 65 instead of
  512) with a ones-column appended to V so the softmax denominator falls out
  of the PE accumulation for free; normalization is folded into the
  PSUM->SBUF copy. Removes all denominator adds and broadcast-back matmuls.
- Attention heads are concatenated in [s, he] layout (hC) and DMA-crossbar
  transposed per s-tile into hT for the out-projection.
- Scores accumulate in bf16 PSUM so an exp instruction spans 2 t-tiles
  (2048 elements), halving Act-engine instruction overhead.

LayerNorm gamma/beta are folded into the QKV weights/biases on the host
(exact: projections are linear in xn). The 1/sqrt(E) score scale is folded
into Wq. Matmul operands are bf16 with fp32 PSUM accumulation for
projections/PV/out-proj; LN statistics and the residual path stay fp32.
"""

import numpy as np
import ml_dtypes

import concourse.bass as bass
import concourse.mybir as mybir
import concourse.tile as tile
from concourse import bacc
from concourse import bass_utils
from concourse.bass import ts

BF_NP = ml_dtypes.bfloat16

B, S, D = 4, 2048, 1024
H, E = 16, 64
LN_EPS = 1e-5
SCALE = 8.0  # sqrt(E) * TEMP

N_CORES = 8
HL = H // 2          # heads per core
NP_ = HL // 2        # 4 head pairs per core
KT = D // 128        # 8 contraction tiles for D
ST = S // 128        # 16 s-tiles of 128
NB = S // 512        # 4 s-blocks of 512
TT = S // 128        # 16 t-tiles of 128

F32 = mybir.dt.float32
BF = mybir.dt.bfloat16

# Score PSUM dtype: bf16 lets one exp instruction span TSPAN=2 t-tiles within
# the PSUM budget. Fallback (if bf16 matmul-out to PSUM misbehaves):
# F32S=True -> f32 scores, TSPAN=1.
F32S = True
TSPAN = 1 if F32S else 2
GT = TT // TSPAN     # groups per block
SDT = F32 if F32S else BF

_NC_CACHE = None


def _emit(nc, aps):
    x_ap = aps["x"]
    xr_ap = aps["xr"]
    wq_ap, wk_ap, wv_ap, wo_ap = aps["wq"], aps["wk"], aps["wv"], aps["wo"]
    bq_ap, bk_ap = aps["bq"], aps["bk"]
    out_ap = aps["out"]

    tc = aps["tc"]
    import contextlib

    ctx = contextlib.ExitStack()
    with ctx:
        const = ctx.enter_context(tc.tile_pool(name="const", bufs=1))
        big = ctx.enter_context(tc.tile_pool(name="big", bufs=1))
        xin = ctx.enter_context(tc.tile_pool(name="xin", bufs=12))
        stat = ctx.enter_context(tc.tile_pool(name="stat", bufs=8))
        xnp = ctx.enter_context(tc.tile_pool(name="xnp", bufs=6))
        ptp = ctx.enter_context(tc.tile_pool(name="ptp", bufs=3))
        r8p = ctx.enter_context(tc.tile_pool(name="r8p", bufs=2))
        xrp = ctx.enter_context(tc.tile_pool(name="xrp", bufs=2))
        outp = ctx.enter_context(tc.tile_pool(name="outp", bufs=4))
        psA = ctx.enter_context(tc.tile_pool(name="psA", bufs=2, space="PSUM"))
        psS = ctx.enter_context(tc.tile_pool(name="psS", bufs=2, space="PSUM"))
        psB = ctx.enter_context(tc.tile_pool(name="psB", bufs=1, space="PSUM"))

        # ---- constants / weights resident in SBUF ----
        # wv/wk ride the fast HWDGE (sync) queue right after the first x
        # tiles: V(0) needs wv at ~7us, K(0,0) needs wk at ~10us. The SWDGE
        # (gpsimd) path costs ~1us Pool-engine time per DMA, which starved
        # the projections when all weights went through it.
        wq_sb = const.tile([128, KT, 512], BF, tag="wq")
        wk_sb = const.tile([128, KT, 512], BF, tag="wk")
        wv_sb = const.tile([128, KT, 512], BF, tag="wv")
        wo_sb = const.tile([128, 4, 1024], BF, tag="wo")
        bq_sb = const.tile([128, NP_], F32, tag="bq")
        bk_sb = const.tile([128, NP_], F32, tag="bk")
        # all weights ride the gpsimd SWDGE queue (Pool engine is otherwise
        # idle), most-urgent first: wv for V(0) at ~9us, wk for K(p0),
        # wq/wo much later. Keeps both HWDGE queues free for x tiles and
        # transposes. bv is folded into xr on the host (exact: the softmax
        # rows sum to one, so +bv passes through attention untouched).
        nc.gpsimd.dma_start(out=bq_sb, in_=bq_ap)
        nc.gpsimd.dma_start(out=bk_sb, in_=bk_ap)
        for k in range(KT):
            nc.gpsimd.dma_start(out=wv_sb[:, k, :], in_=wv_ap[k])
        for k in range(KT):
            nc.gpsimd.dma_start(out=wk_sb[:, k, :], in_=wk_ap[k])
        for k in range(KT):
            nc.gpsimd.dma_start(out=wq_sb[:, k, :], in_=wq_ap[k])
        for k in range(4):
            nc.gpsimd.dma_start(out=wo_sb[:, k, :], in_=wo_ap[k])
        eps_t = const.tile([128, 1], F32, tag="eps")
        nc.vector.memset(eps_t, LN_EPS)

        # [d%128, s_tile, d//128, s%128] LN(x) transposed, per-tile contiguous
        xnT = big.tile([128, ST, KT, 128], BF, tag="xnT")
        qT = big.tile([128, NP_, S], BF, tag="qT")      # [(pairhead,e), s]
        kT_ = big.tile([128, NP_, S], BF, tag="kT")
        v_sb = big.tile([128, TT, HL, E + 1], BF, tag="v")  # [t, h, e | ones]
        hC = big.tile([128, ST, 512], BF, tag="hC")     # [s, (h,e)] attn out
        hT = big.tile([128, ST, 4, 128], BF, tag="hT")  # [(he)%128, s_tile, (he)//128, s%128]
        # ones column of V' (denominator accumulator feed)
        nc.gpsimd.memset(v_sb[:, :, :, E : E + 1], 1.0)

        # ---- work units ----
        # Phase-A PSUM drains are deferred to the next tile so no in-order
        # DVE instruction ever waits on the transpose->matmul chain (a DVE
        # stall there blocks the next tile's LN stats behind it).
        drains = []

        def flush_drains(keep=0):
            while len(drains) > keep:
                drains.pop(0)()

        def emit_qk(kind, p, n, defer=False):
            w_sb, b_sb, dst = (
                (wq_sb, bq_sb, qT) if kind == "q" else (wk_sb, bk_sb, kT_)
            )
            ps = psA.tile([128, 512], F32, tag="ps", name=f"proj_{kind}_{p}_{n}")
            for k in range(KT):
                nc.tensor.matmul(
                    ps, lhsT=w_sb[:, k, ts(p, 128)],
                    rhs=xnT[:, 4 * n : 4 * n + 4, k, :],
                    start=(k == 0), stop=(k == KT - 1),
                )

            def drain_act():
                # per-partition bias add on the Act engine (phase A: DVE is
                # saturated by the LN chain, Act is idle)
                nc.scalar.activation(
                    out=dst[:, p, ts(n, 512)], in_=ps,
                    func=mybir.ActivationFunctionType.Identity,
                    bias=b_sb[:, p : p + 1],
                )

            def drain_dve():
                nc.vector.tensor_scalar_add(
                    out=dst[:, p, ts(n, 512)], in0=ps, scalar1=b_sb[:, p : p + 1]
                )

            if defer:
                drains.append(drain_act)
            else:
                drain_dve()

        def emit_v(t):
            ps = psA.tile([128, HL, E], F32, tag="ps", name=f"proj_v_{t}")
            for k in range(KT):
                nc.tensor.matmul(
                    ps, lhsT=xnT[:, t, k, :], rhs=wv_sb[:, k, :],
                    start=(k == 0), stop=(k == KT - 1),
                )
            drains.append(
                lambda: nc.scalar.copy(out=v_sb[:, t, :, 0:E], in_=ps)
            )

        def emit_scores(p, n, g):
            s12 = psS.tile([128, TSPAN, 2, 512], SDT, tag="s12",
                           name=f"s12_{p}_{n}_{g}")
            for tt in range(TSPAN):
                t = TSPAN * g + tt
                nc.tensor.matmul(
                    s12[:, tt, 0, :], lhsT=kT_[0:64, p, ts(t, 128)],
                    rhs=qT[0:64, p, ts(n, 512)],
                    start=True, stop=True, tile_position=(0, 0),
                )
                nc.tensor.matmul(
                    s12[:, tt, 1, :], lhsT=kT_[64:128, p, ts(t, 128)],
                    rhs=qT[64:128, p, ts(n, 512)],
                    start=True, stop=True, tile_position=(64, 0),
                )
            return s12

        def emit_out_tile(i):
            xr_t = xrp.tile([128, D], F32, tag="xr", name=f"xr_{i}")
            nc.gpsimd.dma_start(out=xr_t, in_=xr_ap[ts(i, 128), :])
            for c in range(2):
                ps_o = psA.tile([128, 512], F32, tag="ps", name=f"pso_{i}_{c}")
                for k in range(4):
                    nc.tensor.matmul(
                        ps_o, lhsT=hT[:, i, k, :], rhs=wo_sb[:, k, ts(c, 512)],
                        start=(k == 0), stop=(k == 3),
                    )
                osb = outp.tile([128, 512], F32, tag="ob", name=f"ob_{i}_{c}")
                nc.vector.tensor_add(out=osb, in0=ps_o, in1=xr_t[:, ts(c, 512)])
                nc.gpsimd.dma_start(out=out_ap[ts(i, 128), ts(c, 512)], in_=osb)

        # ---- phase A: LN + transpose-DMA; V and pair-0 K/Q inline ----
        x_tiles = {}

        def issue_x(i):
            if i < ST and i not in x_tiles:
                x_tiles[i] = xin.tile([128, D], BF, tag="x", name=f"x_{i}")
                nc.sync.dma_start(out=x_tiles[i], in_=x_ap[ts(i, 128), :])

        for i in range(12):
            issue_x(i)
        for i in range(ST):
            x_t = x_tiles.pop(i)
            stats = stat.tile([128, 2, 6], F32, tag="st")
            for sg in range(2):
                nc.vector.bn_stats(out=stats[:, sg, :], in_=x_t[:, ts(sg, 512)])
            mv = stat.tile([128, 2], F32, tag="mv")
            nc.vector.bn_aggr(out=mv, in_=stats)
            std = stat.tile([128, 1], F32, tag="sd")
            nc.scalar.activation(
                out=std, in_=mv[:, 1:2],
                func=mybir.ActivationFunctionType.Sqrt, bias=eps_t,
            )
            istd = stat.tile([128, 1], F32, tag="is")
            nc.vector.reciprocal(out=istd, in_=std)
            xn_t = xnp.tile([128, D], BF, tag="xn")
            nc.vector.tensor_scalar(
                out=xn_t, in0=x_t,
                scalar1=mv[:, 0:1], scalar2=istd,
                op0=mybir.AluOpType.subtract, op1=mybir.AluOpType.mult,
            )
            nc.sync.dma_start_transpose(xnT[:, i], xn_t)
            issue_x(i + 12)
            flush_drains(keep=1)
            if i >= 1:
                emit_v(i - 1)
            if i % 4 == 0 and i > 0:
                emit_qk("k", 0, i // 4 - 1, defer=True)
        emit_v(ST - 1)
        emit_qk("k", 0, NB - 1, defer=True)
        emit_qk("q", 0, 0, defer=True)
        flush_drains()

        # later pairs' projections stream into the Act-bound attention window
        work_queue = (
            [("q", 0, n) for n in range(1, NB)]
            + [(kind, p, n) for p in range(1, NP_)
               for kind in ("k", "q") for n in range(NB)]
        )

        def pop_work():
            if work_queue:
                emit_qk(*work_queue.pop(0))

        if TSPAN == 2:
            pop_slots = (2, 5)
            out_slots = (1, 3, 5, 7)
        else:
            pop_slots = (5, 11)
            out_slots = (2, 6, 10, 14)

        # ---- phase B: attention per (head-pair, s-block) ----
        blocks = [(p, n) for p in range(NP_) for n in range(NB)]
        s12_next = emit_scores(0, 0, 0)
        for bi, (p, n) in enumerate(blocks):
            psv = [
                psB.tile([128, 4, E + 1], F32, tag=f"pv{h}", name=f"pv_{p}_{n}_{h}")
                for h in range(2)
            ]
            for g in range(GT):
                s12 = s12_next
                pt = ptp.tile([128, TSPAN, 2, 512], BF, tag="pt",
                              name=f"pt_{p}_{n}_{g}")
                nc.scalar.activation(
                    out=pt, in_=s12, func=mybir.ActivationFunctionType.Exp
                )
                # prefetch next scores tile so PE stays busy during exp
                if g < GT - 1:
                    s12_next = emit_scores(p, n, g + 1)
                elif bi < len(blocks) - 1:
                    s12_next = emit_scores(*blocks[bi + 1], 0)
                if g in pop_slots:
                    pop_work()
                if p == NP_ - 1 and n >= 1 and g in out_slots:
                    emit_out_tile(4 * (n - 1) + out_slots.index(g))
                for tt in range(TSPAN):
                    t = TSPAN * g + tt
                    for h in range(2):
                        hg = 2 * p + h
                        for sc in range(4):
                            # one start/stop per PSUM bank: the sc==0 start
                            # zeroes the whole 2KB region for all 4 slots
                            nc.tensor.matmul(
                                psv[h][:, sc, :],
                                lhsT=pt[:, tt, h, ts(sc, 128)],
                                rhs=v_sb[:, t, hg, :],
                                start=(t == 0 and sc == 0),
                                stop=(t == TT - 1 and sc == 3),
                                skip_group_check=True,
                            )
            # block epilogue: normalize into hC (denominator is psv[..., E])
            r8 = r8p.tile([128, HL], F32, tag="r8", name=f"r8_{p}_{n}")
            for h in range(2):
                nc.vector.reciprocal(out=r8[:, 4 * h : 4 * h + 4], in_=psv[h][:, :, E])
            for j in range(HL):
                h, sc = j // 4, j % 4
                nc.vector.tensor_scalar_mul(
                    out=hC[:, n * 4 + sc, ts(2 * p + h, E)],
                    in0=psv[h][:, sc, 0:E],
                    scalar1=r8[:, j : j + 1],
                )
            if p == NP_ - 1:
                for sc in range(4):
                    i = n * 4 + sc
                    nc.sync.dma_start_transpose(hT[:, i], hC[:, i, :])
        # remaining out-projection tiles
        for i in range(4 * (NB - 1), ST):
            emit_out_tile(i)

        aps["_dbg"] = {
            "xnT": xnT.tensor.name, "qT": qT.tensor.name, "kT_": kT_.tensor.name,
            "v_sb": v_sb.tensor.name, "hC": hC.tensor.name, "hT": hT.tensor.name,
        }


def build():
    nc = bacc.Bacc("TRN2", target_bir_lowering=False, debug=False, num_devices=N_CORES)
    aps = {
        "x": nc.dram_tensor("x", [S, D], BF, kind="ExternalInput").ap(),
        "xr": nc.dram_tensor("xr", [S, D], F32, kind="ExternalInput").ap(),
        "wq": nc.dram_tensor("wq", [KT, 128, 512], BF, kind="ExternalInput").ap(),
        "wk": nc.dram_tensor("wk", [KT, 128, 512], BF, kind="ExternalInput").ap(),
        "wv": nc.dram_tensor("wv", [KT, 128, 512], BF, kind="ExternalInput").ap(),
        "wo": nc.dram_tensor("wo", [4, 128, 1024], BF, kind="ExternalInput").ap(),
        "bq": nc.dram_tensor("bq", [128, NP_], F32, kind="ExternalInput").ap(),
        "bk": nc.dram_tensor("bk", [128, NP_], F32, kind="ExternalInput").ap(),
        "out": nc.dram_tensor("out", [S, D], F32, kind="ExternalOutput").ap(),
    }
    with tile.TileContext(nc) as tc:
        aps["tc"] = tc
        _emit(nc, aps)
    nc._dbg_names = aps.get("_dbg", {})
    nc.compile()
    return nc


def prep_core_inputs(x, Wq, bq, Wk, bk, Wv, bv, Wo, bo, ln_gamma, ln_beta):
    """Host-side sharding: returns list of 8 in_maps (numpy arrays)."""
    x = np.asarray(x, np.float32)
    Wq, bq = np.asarray(Wq, np.float32), np.asarray(bq, np.float32)
    Wk, bk = np.asarray(Wk, np.float32), np.asarray(bk, np.float32)
    Wv, bv = np.asarray(Wv, np.float32), np.asarray(bv, np.float32)
    Wo, bo = np.asarray(Wo, np.float32), np.asarray(bo, np.float32)
    gamma, beta = np.asarray(ln_gamma, np.float32), np.asarray(ln_beta, np.float32)

    Wq_eff = Wq * gamma[None, None, :] / SCALE
    bq_eff = (bq + Wq @ beta) / SCALE
    Wk_eff = Wk * gamma[None, None, :]
    bk_eff = bk + Wk @ beta
    Wv_eff = Wv * gamma[None, None, :]
    bv_eff = bv + Wv @ beta

    def wq_layout(w):  # [8, 64, 1024] -> [KT, 128, 512]
        # w[h, e, kt*128+dd] -> out[kt, dd, h*64+e]
        return np.ascontiguousarray(
            w.reshape(HL * E, KT, 128).transpose(1, 2, 0)
        ).astype(BF_NP)

    def b_layout(b):  # [8, 64] -> [128, 4]: out[(hh*64+e), p] = b[2p+hh, e]
        return np.ascontiguousarray(
            b.reshape(NP_, 2 * E).T
        ).astype(np.float32)

    in_maps = []
    for c in range(N_CORES):
        bidx, g = c // 2, c % 2
        hs = slice(g * HL, (g + 1) * HL)
        wo_loc = Wo[:, g * 512:(g + 1) * 512]  # [1024, 512]
        wo_dev = np.ascontiguousarray(
            wo_loc.T.reshape(4, 128, 1024)
        ).astype(BF_NP)
        bv_l = bv_eff[hs].reshape(512)
        in_maps.append({
            "x": x[bidx].astype(BF_NP),
            "xr": 0.5 * (x[bidx] + bo[None, :]) + (wo_loc.astype(np.float32) @ bv_l)[None, :],
            "wq": wq_layout(Wq_eff[hs]),
            "wk": wq_layout(Wk_eff[hs]),
            "wv": wq_layout(Wv_eff[hs]),
            "wo": wo_dev,
            "bq": b_layout(bq_eff[hs]),
            "bk": b_layout(bk_eff[hs]),
            "out": np.zeros((S, D), np.float32),
        })
    return in_maps


def kernel(x, Wq, bq, Wk, bk, Wv, bv, Wo, bo, ln_gamma, ln_beta):
    global _NC_CACHE
    if _NC_CACHE is None:
        _NC_CACHE = build()
    nc = _NC_CACHE
    in_maps = prep_core_inputs(x, Wq, bq, Wk, bk, Wv, bv, Wo, bo, ln_gamma, ln_beta)
    for m in in_maps:
        m.pop("out")
    res = bass_utils.run_bass_kernel_spmd(nc, in_maps, core_ids=list(range(N_CORES)))
    out = np.empty((B, S, D), np.float32)
    for bidx in range(B):
        out[bidx] = res.results[2 * bidx]["out"] + res.results[2 * bidx + 1]["out"]
    return out


# revision 22
# speedup vs baseline: 1.0872x; 1.0542x over previous
"""Multi-head attention block (pre-LN, residual) on 8 Trainium2 NeuronCores.

Sharding: (batch x head-group) grid. Core c handles batch b = c//2 and head
group g = c%2 (8 of 16 heads). Per core: LN(x_b) -> per-head QKV projections
-> softmax attention (no max-subtraction; scores are O(10)) -> out-projection
against the local 512-wide slice of Wo, + 0.5*(x+bo) residual. Host sums the
two partial outputs per batch (the pair all-reduce) and stacks batches.

v2 structure:
- LN apply moved to the Act engine (Identity activation with per-partition
  scale/bias), stats stay on DVE.
- xn -> xnT via DMA crossbar transposes (one per s-tile) instead of PE
  identity-matmul transposes + vector copies.
- PV matmuls flipped to [s, e] output orientation (free size 65 instead of
  512) with a ones-column appended to V so the softmax denominator falls out
  of the PE accumulation for free; normalization is folded into the
  PSUM->SBUF copy. Removes all denominator adds and broadcast-back matmuls.
- Attention heads are concatenated in [s, he] layout (hC) and DMA-crossbar
  transposed per s-tile into hT for the out-projection.
- Scores accumulate in bf16 PSUM so an exp instruction spans 2 t-tiles
  (2048 elements), halving Act-engine instruction overhead.

LayerNorm gamma/beta are folded into the QKV weights/biases on the host
(exact: projections are linear in xn). The 1/sqrt(E) score scale is folded
into Wq. Matmul operands are bf16 with fp32 PSUM accumulation for
projections/PV/out-proj; LN statistics and the residual path stay fp32.
"""

import numpy as np
import ml_dtypes

import concourse.bass as bass
import concourse.mybir as mybir
import concourse.tile as tile
from concourse import bacc
from concourse import bass_utils
from concourse.bass import ts

BF_NP = ml_dtypes.bfloat16

B, S, D = 4, 2048, 1024
H, E = 16, 64
LN_EPS = 1e-5
SCALE = 8.0  # sqrt(E) * TEMP

N_CORES = 8
HL = H // 2          # heads per core
NP_ = HL // 2        # 4 head pairs per core
KT = D // 128        # 8 contraction tiles for D
ST = S // 128        # 16 s-tiles of 128
NB = S // 512        # 4 s-blocks of 512
TT = S // 128        # 16 t-tiles of 128

F32 = mybir.dt.float32
BF = mybir.dt.bfloat16

# Score PSUM dtype: bf16 lets one exp instruction span TSPAN=2 t-tiles within
# the PSUM budget. Fallback (if bf16 matmul-out to PSUM misbehaves):
# F32S=True -> f32 scores, TSPAN=1.
F32S = True
TSPAN = 1 if F32S else 2
GT = TT // TSPAN     # groups per block
SDT = F32 if F32S else BF

_NC_CACHE = None


def _emit(nc, aps):
    x_ap = aps["x"]
    xr_ap = aps["xr"]
    wq_ap, wk_ap, wv_ap, wo_ap = aps["wq"], aps["wk"], aps["wv"], aps["wo"]
    bq_ap, bk_ap = aps["bq"], aps["bk"]
    out_ap = aps["out"]

    tc = aps["tc"]
    import contextlib

    ctx = contextlib.ExitStack()
    with ctx:
        const = ctx.enter_context(tc.tile_pool(name="const", bufs=1))
        big = ctx.enter_context(tc.tile_pool(name="big", bufs=1))
        xin = ctx.enter_context(tc.tile_pool(name="xin", bufs=12))
        stat = ctx.enter_context(tc.tile_pool(name="stat", bufs=8))
        xnp = ctx.enter_context(tc.tile_pool(name="xnp", bufs=6))
        ptp = ctx.enter_context(tc.tile_pool(name="ptp", bufs=3))
        r8p = ctx.enter_context(tc.tile_pool(name="r8p", bufs=2))
        xrp = ctx.enter_context(tc.tile_pool(name="xrp", bufs=2))
        outp = ctx.enter_context(tc.tile_pool(name="outp", bufs=4))
        psA = ctx.enter_context(tc.tile_pool(name="psA", bufs=2, space="PSUM"))
        psS = ctx.enter_context(tc.tile_pool(name="psS", bufs=2, space="PSUM"))
        psB = ctx.enter_context(tc.tile_pool(name="psB", bufs=1, space="PSUM"))

        # ---- constants / weights resident in SBUF ----
        # wv/wk ride the fast HWDGE (sync) queue right after the first x
        # tiles: V(0) needs wv at ~7us, K(0,0) needs wk at ~10us. The SWDGE
        # (gpsimd) path costs ~1us Pool-engine time per DMA, which starved
        # the projections when all weights went through it.
        wq_sb = const.tile([128, KT, 512], BF, tag="wq")
        wk_sb = const.tile([128, KT, 512], BF, tag="wk")
        wv_sb = const.tile([128, KT, 512], BF, tag="wv")
        wo_sb = const.tile([128, 4, 1024], BF, tag="wo")
        bq_sb = const.tile([128, NP_], F32, tag="bq")
        bk_sb = const.tile([128, NP_], F32, tag="bk")
        # all weights ride the gpsimd SWDGE queue (Pool engine is otherwise
        # idle), most-urgent first: wv for V(0) at ~9us, wk for K(p0),
        # wq/wo much later. Keeps both HWDGE queues free for x tiles and
        # transposes. bv is folded into xr on the host (exact: the softmax
        # rows sum to one, so +bv passes through attention untouched).
        nc.gpsimd.dma_start(out=bq_sb, in_=bq_ap)
        nc.gpsimd.dma_start(out=bk_sb, in_=bk_ap)
        for k in range(KT):
            nc.gpsimd.dma_start(out=wv_sb[:, k, :], in_=wv_ap[k])
        for k in range(KT):
            nc.gpsimd.dma_start(out=wk_sb[:, k, :], in_=wk_ap[k])
        for k in range(KT):
            nc.gpsimd.dma_start(out=wq_sb[:, k, :], in_=wq_ap[k])
        for k in range(4):
            nc.gpsimd.dma_start(out=wo_sb[:, k, :], in_=wo_ap[k])
        eps_t = const.tile([128, 1], F32, tag="eps")
        nc.vector.memset(eps_t, LN_EPS)

        # [d%128, s_tile, d//128, s%128] LN(x) transposed, per-tile contiguous
        xnT = big.tile([128, ST, KT, 128], BF, tag="xnT")
        qT = big.tile([128, NP_, S], BF, tag="qT")      # [(pairhead,e), s]
        kT_ = big.tile([128, NP_, S], BF, tag="kT")
        v_sb = big.tile([128, TT, HL, E + 1], BF, tag="v")  # [t, h, e | ones]
        hC = big.tile([128, ST, 512], BF, tag="hC")     # [s, (h,e)] attn out
        hT = big.tile([128, ST, 4, 128], BF, tag="hT")  # [(he)%128, s_tile, (he)//128, s%128]
        # ones column of V' (denominator accumulator feed)
        nc.gpsimd.memset(v_sb[:, :, :, E : E + 1], 1.0)

        # ---- work units ----
        # Phase-A PSUM drains are deferred to the next tile so no in-order
        # DVE instruction ever waits on the transpose->matmul chain (a DVE
        # stall there blocks the next tile's LN stats behind it).
        drains = []

        def flush_drains(keep=0):
            while len(drains) > keep:
                drains.pop(0)()

        def emit_qk(kind, p, n, defer=False):
            # never leave >1 undrained psA unit: the ring has 2 slots and a
            # writer emitted before the previous reader is a race
            flush_drains(keep=1)
            w_sb, b_sb, dst = (
                (wq_sb, bq_sb, qT) if kind == "q" else (wk_sb, bk_sb, kT_)
            )
            ps = psA.tile([128, 512], F32, tag="ps", name=f"proj_{kind}_{p}_{n}")
            for k in range(KT):
                nc.tensor.matmul(
                    ps, lhsT=w_sb[:, k, ts(p, 128)],
                    rhs=xnT[:, 4 * n : 4 * n + 4, k, :],
                    start=(k == 0), stop=(k == KT - 1),
                )

            def drain_act():
                # per-partition bias add on the Act engine (phase A: DVE is
                # saturated by the LN chain, Act is idle)
                nc.scalar.activation(
                    out=dst[:, p, ts(n, 512)], in_=ps,
                    func=mybir.ActivationFunctionType.Identity,
                    bias=b_sb[:, p : p + 1],
                )

            def drain_dve():
                nc.vector.tensor_scalar_add(
                    out=dst[:, p, ts(n, 512)], in0=ps, scalar1=b_sb[:, p : p + 1]
                )

            if defer:
                drains.append(drain_act)
            else:
                drain_dve()

        def emit_v(t):
            flush_drains(keep=1)
            ps = psA.tile([128, HL, E], F32, tag="ps", name=f"proj_v_{t}")
            for k in range(KT):
                nc.tensor.matmul(
                    ps, lhsT=xnT[:, t, k, :], rhs=wv_sb[:, k, :],
                    start=(k == 0), stop=(k == KT - 1),
                )
            drains.append(
                lambda: nc.scalar.copy(out=v_sb[:, t, :, 0:E], in_=ps)
            )

        def emit_scores(p, n, g):
            s12 = psS.tile([128, TSPAN, 2, 512], SDT, tag="s12",
                           name=f"s12_{p}_{n}_{g}")
            for tt in range(TSPAN):
                t = TSPAN * g + tt
                nc.tensor.matmul(
                    s12[:, tt, 0, :], lhsT=kT_[0:64, p, ts(t, 128)],
                    rhs=qT[0:64, p, ts(n, 512)],
                    start=True, stop=True, tile_position=(0, 0),
                )
                nc.tensor.matmul(
                    s12[:, tt, 1, :], lhsT=kT_[64:128, p, ts(t, 128)],
                    rhs=qT[64:128, p, ts(n, 512)],
                    start=True, stop=True, tile_position=(64, 0),
                )
            return s12

        def emit_out_tile(i):
            xr_t = xrp.tile([128, D], F32, tag="xr", name=f"xr_{i}")
            nc.gpsimd.dma_start(out=xr_t, in_=xr_ap[ts(i, 128), :])
            for c in range(2):
                ps_o = psA.tile([128, 512], F32, tag="ps", name=f"pso_{i}_{c}")
                for k in range(4):
                    nc.tensor.matmul(
                        ps_o, lhsT=hT[:, i, k, :], rhs=wo_sb[:, k, ts(c, 512)],
                        start=(k == 0), stop=(k == 3),
                    )
                osb = outp.tile([128, 512], F32, tag="ob", name=f"ob_{i}_{c}")
                nc.vector.tensor_add(out=osb, in0=ps_o, in1=xr_t[:, ts(c, 512)])
                nc.gpsimd.dma_start(out=out_ap[ts(i, 128), ts(c, 512)], in_=osb)

        # ---- phase A: LN + transpose-DMA; V and pair-0 K/Q inline ----
        x_tiles = {}

        def issue_x(i):
            if i < ST and i not in x_tiles:
                x_tiles[i] = xin.tile([128, D], BF, tag="x", name=f"x_{i}")
                nc.sync.dma_start(out=x_tiles[i], in_=x_ap[ts(i, 128), :])

        for i in range(12):
            issue_x(i)
        for i in range(ST):
            x_t = x_tiles.pop(i)
            stats = stat.tile([128, 2, 6], F32, tag="st")
            for sg in range(2):
                nc.vector.bn_stats(out=stats[:, sg, :], in_=x_t[:, ts(sg, 512)])
            mv = stat.tile([128, 2], F32, tag="mv")
            nc.vector.bn_aggr(out=mv, in_=stats)
            std = stat.tile([128, 1], F32, tag="sd")
            nc.scalar.activation(
                out=std, in_=mv[:, 1:2],
                func=mybir.ActivationFunctionType.Sqrt, bias=eps_t,
            )
            istd = stat.tile([128, 1], F32, tag="is")
            nc.vector.reciprocal(out=istd, in_=std)
            xn_t = xnp.tile([128, D], BF, tag="xn")
            nc.vector.tensor_scalar(
                out=xn_t, in0=x_t,
                scalar1=mv[:, 0:1], scalar2=istd,
                op0=mybir.AluOpType.subtract, op1=mybir.AluOpType.mult,
            )
            nc.sync.dma_start_transpose(xnT[:, i], xn_t)
            issue_x(i + 12)
            flush_drains(keep=1)
            if i >= 1:
                emit_v(i - 1)
            if i % 4 == 0 and i > 0:
                emit_qk("k", 0, i // 4 - 1, defer=True)
                emit_qk("k", 1, i // 4 - 1, defer=True)
        emit_v(ST - 1)
        emit_qk("k", 0, NB - 1, defer=True)
        emit_qk("k", 1, NB - 1, defer=True)
        emit_qk("q", 0, 0, defer=True)
        flush_drains()

        # remaining projections stream into the Act-bound attention window,
        # ordered by deadline under the blocks-outer schedule: block (n, p)
        # needs K(p, all) and Q(p, n) before it starts.
        work_queue = (
            [("q", 1, 0)]
            + [("k", 2, n) for n in range(NB)] + [("q", 2, 0)]
            + [("k", 3, 0), ("q", 3, 0), ("k", 3, 1), ("k", 3, 2), ("k", 3, 3)]
            + [("q", p, n) for n in range(1, NB) for p in range(NP_)]
        )

        def pop_work():
            if work_queue:
                emit_qk(*work_queue.pop(0))

        if TSPAN == 2:
            pop_slots = (1, 4, 6)
            out_slots = (3, 7)
        else:
            pop_slots = (3, 8, 13)
            out_slots = (6, 15)

        # ---- phase B: attention, blocks outer so each s-block's output
        # completes early and the out-projection streams across the run ----
        blocks = [(p, n) for n in range(NB) for p in range(NP_)]
        ready_out = []
        s12_next = emit_scores(0, 0, 0)
        for bi, (p, n) in enumerate(blocks):
            psv = [
                psB.tile([128, 4, E + 1], F32, tag=f"pv{h}", name=f"pv_{p}_{n}_{h}")
                for h in range(2)
            ]
            for g in range(GT):
                s12 = s12_next
                pt = ptp.tile([128, TSPAN, 2, 512], BF, tag="pt",
                              name=f"pt_{p}_{n}_{g}")
                nc.scalar.activation(
                    out=pt, in_=s12, func=mybir.ActivationFunctionType.Exp
                )
                # prefetch next scores tile so PE stays busy during exp
                if g < GT - 1:
                    s12_next = emit_scores(p, n, g + 1)
                elif bi < len(blocks) - 1:
                    s12_next = emit_scores(*blocks[bi + 1], 0)
                if g in pop_slots:
                    pop_work()
                if g in out_slots and ready_out:
                    emit_out_tile(ready_out.pop(0))
                for tt in range(TSPAN):
                    t = TSPAN * g + tt
                    for h in range(2):
                        hg = 2 * p + h
                        for sc in range(4):
                            # one start/stop per PSUM bank: the sc==0 start
                            # zeroes the whole 2KB region for all 4 slots
                            nc.tensor.matmul(
                                psv[h][:, sc, :],
                                lhsT=pt[:, tt, h, ts(sc, 128)],
                                rhs=v_sb[:, t, hg, :],
                                start=(t == 0 and sc == 0),
                                stop=(t == TT - 1 and sc == 3),
                                skip_group_check=True,
                            )
            # block epilogue: normalize into hC (denominator is psv[..., E])
            r8 = r8p.tile([128, HL], F32, tag="r8", name=f"r8_{p}_{n}")
            for h in range(2):
                nc.vector.reciprocal(out=r8[:, 4 * h : 4 * h + 4], in_=psv[h][:, :, E])
            for j in range(HL):
                h, sc = j // 4, j % 4
                nc.vector.tensor_scalar_mul(
                    out=hC[:, n * 4 + sc, ts(2 * p + h, E)],
                    in0=psv[h][:, sc, 0:E],
                    scalar1=r8[:, j : j + 1],
                )
            if p == NP_ - 1:
                for sc in range(4):
                    i = n * 4 + sc
                    nc.sync.dma_start_transpose(hT[:, i], hC[:, i, :])
                ready_out.extend(range(n * 4, n * 4 + 4))
        # remaining out-projection tiles
        while ready_out:
            emit_out_tile(ready_out.pop(0))

        aps["_dbg"] = {
            "xnT": xnT.tensor.name, "qT": qT.tensor.name, "kT_": kT_.tensor.name,
            "v_sb": v_sb.tensor.name, "hC": hC.tensor.name, "hT": hT.tensor.name,
        }


def build():
    nc = bacc.Bacc("TRN2", target_bir_lowering=False, debug=False, num_devices=N_CORES)
    aps = {
        "x": nc.dram_tensor("x", [S, D], BF, kind="ExternalInput").ap(),
        "xr": nc.dram_tensor("xr", [S, D], F32, kind="ExternalInput").ap(),
        "wq": nc.dram_tensor("wq", [KT, 128, 512], BF, kind="ExternalInput").ap(),
        "wk": nc.dram_tensor("wk", [KT, 128, 512], BF, kind="ExternalInput").ap(),
        "wv": nc.dram_tensor("wv", [KT, 128, 512], BF, kind="ExternalInput").ap(),
        "wo": nc.dram_tensor("wo", [4, 128, 1024], BF, kind="ExternalInput").ap(),
        "bq": nc.dram_tensor("bq", [128, NP_], F32, kind="ExternalInput").ap(),
        "bk": nc.dram_tensor("bk", [128, NP_], F32, kind="ExternalInput").ap(),
        "out": nc.dram_tensor("out", [S, D], F32, kind="ExternalOutput").ap(),
    }
    with tile.TileContext(nc) as tc:
        aps["tc"] = tc
        _emit(nc, aps)
    nc._dbg_names = aps.get("_dbg", {})
    nc.compile()
    return nc


def prep_core_inputs(x, Wq, bq, Wk, bk, Wv, bv, Wo, bo, ln_gamma, ln_beta):
    """Host-side sharding: returns list of 8 in_maps (numpy arrays)."""
    x = np.asarray(x, np.float32)
    Wq, bq = np.asarray(Wq, np.float32), np.asarray(bq, np.float32)
    Wk, bk = np.asarray(Wk, np.float32), np.asarray(bk, np.float32)
    Wv, bv = np.asarray(Wv, np.float32), np.asarray(bv, np.float32)
    Wo, bo = np.asarray(Wo, np.float32), np.asarray(bo, np.float32)
    gamma, beta = np.asarray(ln_gamma, np.float32), np.asarray(ln_beta, np.float32)

    Wq_eff = Wq * gamma[None, None, :] / SCALE
    bq_eff = (bq + Wq @ beta) / SCALE
    Wk_eff = Wk * gamma[None, None, :]
    bk_eff = bk + Wk @ beta
    Wv_eff = Wv * gamma[None, None, :]
    bv_eff = bv + Wv @ beta

    def wq_layout(w):  # [8, 64, 1024] -> [KT, 128, 512]
        # w[h, e, kt*128+dd] -> out[kt, dd, h*64+e]
        return np.ascontiguousarray(
            w.reshape(HL * E, KT, 128).transpose(1, 2, 0)
        ).astype(BF_NP)

    def b_layout(b):  # [8, 64] -> [128, 4]: out[(hh*64+e), p] = b[2p+hh, e]
        return np.ascontiguousarray(
            b.reshape(NP_, 2 * E).T
        ).astype(np.float32)

    in_maps = []
    for c in range(N_CORES):
        bidx, g = c // 2, c % 2
        hs = slice(g * HL, (g + 1) * HL)
        wo_loc = Wo[:, g * 512:(g + 1) * 512]  # [1024, 512]
        wo_dev = np.ascontiguousarray(
            wo_loc.T.reshape(4, 128, 1024)
        ).astype(BF_NP)
        bv_l = bv_eff[hs].reshape(512)
        in_maps.append({
            "x": x[bidx].astype(BF_NP),
            "xr": 0.5 * (x[bidx] + bo[None, :]) + (wo_loc.astype(np.float32) @ bv_l)[None, :],
            "wq": wq_layout(Wq_eff[hs]),
            "wk": wq_layout(Wk_eff[hs]),
            "wv": wq_layout(Wv_eff[hs]),
            "wo": wo_dev,
            "bq": b_layout(bq_eff[hs]),
            "bk": b_layout(bk_eff[hs]),
            "out": np.zeros((S, D), np.float32),
        })
    return in_maps


def kernel(x, Wq, bq, Wk, bk, Wv, bv, Wo, bo, ln_gamma, ln_beta):
    global _NC_CACHE
    if _NC_CACHE is None:
        _NC_CACHE = build()
    nc = _NC_CACHE
    in_maps = prep_core_inputs(x, Wq, bq, Wk, bk, Wv, bv, Wo, bo, ln_gamma, ln_beta)
    for m in in_maps:
        m.pop("out")
    res = bass_utils.run_bass_kernel_spmd(nc, in_maps, core_ids=list(range(N_CORES)))
    out = np.empty((B, S, D), np.float32)
    for bidx in range(B):
        out[bidx] = res.results[2 * bidx]["out"] + res.results[2 * bidx + 1]["out"]
    return out


# revision 23
# speedup vs baseline: 1.0952x; 1.0073x over previous
"""Multi-head attention block (pre-LN, residual) on 8 Trainium2 NeuronCores.

Sharding: (batch x head-group) grid. Core c handles batch b = c//2 and head
group g = c%2 (8 of 16 heads). Per core: LN(x_b) -> per-head QKV projections
-> softmax attention (no max-subtraction; scores are O(10)) -> out-projection
against the local 512-wide slice of Wo, + 0.5*(x+bo) residual. Host sums the
two partial outputs per batch (the pair all-reduce) and stacks batches.

v2 structure:
- LN apply moved to the Act engine (Identity activation with per-partition
  scale/bias), stats stay on DVE.
- xn -> xnT via DMA crossbar transposes (one per s-tile) instead of PE
  identity-matmul transposes + vector copies.
- PV matmuls flipped to [s, e] output orientation (free size 65 instead of
  512) with a ones-column appended to V so the softmax denominator falls out
  of the PE accumulation for free; normalization is folded into the
  PSUM->SBUF copy. Removes all denominator adds and broadcast-back matmuls.
- Attention heads are concatenated in [s, he] layout (hC) and DMA-crossbar
  transposed per s-tile into hT for the out-projection.
- Scores accumulate in bf16 PSUM so an exp instruction spans 2 t-tiles
  (2048 elements), halving Act-engine instruction overhead.

LayerNorm gamma/beta are folded into the QKV weights/biases on the host
(exact: projections are linear in xn). The 1/sqrt(E) score scale is folded
into Wq. Matmul operands are bf16 with fp32 PSUM accumulation for
projections/PV/out-proj; LN statistics and the residual path stay fp32.
"""

import numpy as np
import ml_dtypes

import concourse.bass as bass
import concourse.mybir as mybir
import concourse.tile as tile
from concourse import bacc
from concourse import bass_utils
from concourse.bass import ts

BF_NP = ml_dtypes.bfloat16

B, S, D = 4, 2048, 1024
H, E = 16, 64
LN_EPS = 1e-5
SCALE = 8.0  # sqrt(E) * TEMP

N_CORES = 8
HL = H // 2          # heads per core
NP_ = HL // 2        # 4 head pairs per core
KT = D // 128        # 8 contraction tiles for D
ST = S // 128        # 16 s-tiles of 128
NB = S // 512        # 4 s-blocks of 512
TT = S // 128        # 16 t-tiles of 128

F32 = mybir.dt.float32
BF = mybir.dt.bfloat16

# Score PSUM dtype: bf16 lets one exp instruction span TSPAN=2 t-tiles within
# the PSUM budget. Fallback (if bf16 matmul-out to PSUM misbehaves):
# F32S=True -> f32 scores, TSPAN=1.
F32S = True
TSPAN = 1 if F32S else 2
GT = TT // TSPAN     # groups per block
SDT = F32 if F32S else BF

_NC_CACHE = None


def _emit(nc, aps):
    x_ap = aps["x"]
    xr_ap = aps["xr"]
    wq_ap, wk_ap, wv_ap, wo_ap = aps["wq"], aps["wk"], aps["wv"], aps["wo"]
    bq_ap, bk_ap = aps["bq"], aps["bk"]
    out_ap = aps["out"]

    tc = aps["tc"]
    import contextlib

    ctx = contextlib.ExitStack()
    with ctx:
        const = ctx.enter_context(tc.tile_pool(name="const", bufs=1))
        big = ctx.enter_context(tc.tile_pool(name="big", bufs=1))
        xin = ctx.enter_context(tc.tile_pool(name="xin", bufs=12))
        stat = ctx.enter_context(tc.tile_pool(name="stat", bufs=8))
        xnp = ctx.enter_context(tc.tile_pool(name="xnp", bufs=6))
        ptp = ctx.enter_context(tc.tile_pool(name="ptp", bufs=3))
        r8p = ctx.enter_context(tc.tile_pool(name="r8p", bufs=2))
        xrp = ctx.enter_context(tc.tile_pool(name="xrp", bufs=2))
        outp = ctx.enter_context(tc.tile_pool(name="outp", bufs=4))
        psA = ctx.enter_context(tc.tile_pool(name="psA", bufs=2, space="PSUM"))
        # psV is phase-A scoped: its 4 banks are released before the
        # attention pools (psS/psB) are created, so V-projection drains can
        # lag 3 tiles behind their matmuls without blocking the ring.
        psV_cm = tc.tile_pool(name="psV", bufs=4, space="PSUM")
        psV = psV_cm.__enter__()

        # ---- constants / weights resident in SBUF ----
        # wv/wk ride the fast HWDGE (sync) queue right after the first x
        # tiles: V(0) needs wv at ~7us, K(0,0) needs wk at ~10us. The SWDGE
        # (gpsimd) path costs ~1us Pool-engine time per DMA, which starved
        # the projections when all weights went through it.
        wq_sb = const.tile([128, KT, 512], BF, tag="wq")
        wk_sb = const.tile([128, KT, 512], BF, tag="wk")
        wv_sb = const.tile([128, KT, 512], BF, tag="wv")
        wo_sb = const.tile([128, 4, 1024], BF, tag="wo")
        bq_sb = const.tile([128, NP_], F32, tag="bq")
        bk_sb = const.tile([128, NP_], F32, tag="bk")
        # all weights ride the gpsimd SWDGE queue (Pool engine is otherwise
        # idle), most-urgent first: wv for V(0) at ~9us, wk for K(p0),
        # wq/wo much later. Keeps both HWDGE queues free for x tiles and
        # transposes. bv is folded into xr on the host (exact: the softmax
        # rows sum to one, so +bv passes through attention untouched).
        nc.gpsimd.dma_start(out=bq_sb, in_=bq_ap)
        nc.gpsimd.dma_start(out=bk_sb, in_=bk_ap)
        for k in range(KT):
            nc.gpsimd.dma_start(out=wv_sb[:, k, :], in_=wv_ap[k])
        for k in range(KT):
            nc.gpsimd.dma_start(out=wk_sb[:, k, :], in_=wk_ap[k])
        for k in range(KT):
            nc.gpsimd.dma_start(out=wq_sb[:, k, :], in_=wq_ap[k])
        for k in range(4):
            nc.gpsimd.dma_start(out=wo_sb[:, k, :], in_=wo_ap[k])
        eps_t = const.tile([128, 1], F32, tag="eps")
        nc.vector.memset(eps_t, LN_EPS)

        # [d%128, s_tile, d//128, s%128] LN(x) transposed, per-tile contiguous
        xnT = big.tile([128, ST, KT, 128], BF, tag="xnT")
        qT = big.tile([128, NP_, S], BF, tag="qT")      # [(pairhead,e), s]
        kT_ = big.tile([128, NP_, S], BF, tag="kT")
        v_sb = big.tile([128, TT, HL, E + 1], BF, tag="v")  # [t, h, e | ones]
        hC = big.tile([128, ST, 512], BF, tag="hC")     # [s, (h,e)] attn out
        hT = big.tile([128, ST, 4, 128], BF, tag="hT")  # [(he)%128, s_tile, (he)//128, s%128]
        # ones column of V' (denominator accumulator feed)
        nc.gpsimd.memset(v_sb[:, :, :, E : E + 1], 1.0)

        # ---- work units ----
        # Phase-A PSUM drains are deferred to the next tile so no in-order
        # DVE instruction ever waits on the transpose->matmul chain (a DVE
        # stall there blocks the next tile's LN stats behind it).
        drains = []

        def flush_drains(keep=0):
            while len(drains) > keep:
                drains.pop(0)()

        def emit_qk(kind, p, n, defer=False):
            # never leave >1 undrained psA unit: the ring has 2 slots and a
            # writer emitted before the previous reader is a race
            flush_drains(keep=1)
            w_sb, b_sb, dst = (
                (wq_sb, bq_sb, qT) if kind == "q" else (wk_sb, bk_sb, kT_)
            )
            ps = psA.tile([128, 512], F32, tag="ps", name=f"proj_{kind}_{p}_{n}")
            for k in range(KT):
                nc.tensor.matmul(
                    ps, lhsT=w_sb[:, k, ts(p, 128)],
                    rhs=xnT[:, 4 * n : 4 * n + 4, k, :],
                    start=(k == 0), stop=(k == KT - 1),
                )

            def drain_act():
                # per-partition bias add on the Act engine (phase A: DVE is
                # saturated by the LN chain, Act is idle)
                nc.scalar.activation(
                    out=dst[:, p, ts(n, 512)], in_=ps,
                    func=mybir.ActivationFunctionType.Identity,
                    bias=b_sb[:, p : p + 1],
                )

            def drain_dve():
                nc.vector.tensor_scalar_add(
                    out=dst[:, p, ts(n, 512)], in0=ps, scalar1=b_sb[:, p : p + 1]
                )

            if defer:
                drains.append(drain_act)
            else:
                drain_dve()

        v_drains = []

        def flush_v(keep=0):
            while len(v_drains) > keep:
                v_drains.pop(0)()

        def emit_v(t):
            flush_v(keep=3)
            ps = psV.tile([128, HL, E], F32, tag="psv", name=f"proj_v_{t}")
            for k in range(KT):
                nc.tensor.matmul(
                    ps, lhsT=xnT[:, t, k, :], rhs=wv_sb[:, k, :],
                    start=(k == 0), stop=(k == KT - 1),
                )
            v_drains.append(
                lambda: nc.scalar.copy(out=v_sb[:, t, :, 0:E], in_=ps)
            )

        def emit_scores(p, n, g):
            s12 = psS.tile([128, TSPAN, 2, 512], SDT, tag="s12",
                           name=f"s12_{p}_{n}_{g}")
            for tt in range(TSPAN):
                t = TSPAN * g + tt
                nc.tensor.matmul(
                    s12[:, tt, 0, :], lhsT=kT_[0:64, p, ts(t, 128)],
                    rhs=qT[0:64, p, ts(n, 512)],
                    start=True, stop=True, tile_position=(0, 0),
                )
                nc.tensor.matmul(
                    s12[:, tt, 1, :], lhsT=kT_[64:128, p, ts(t, 128)],
                    rhs=qT[64:128, p, ts(n, 512)],
                    start=True, stop=True, tile_position=(64, 0),
                )
            return s12

        def emit_out_tile(i):
            xr_t = xrp.tile([128, D], F32, tag="xr", name=f"xr_{i}")
            nc.gpsimd.dma_start(out=xr_t, in_=xr_ap[ts(i, 128), :])
            for c in range(2):
                ps_o = psA.tile([128, 512], F32, tag="ps", name=f"pso_{i}_{c}")
                for k in range(4):
                    nc.tensor.matmul(
                        ps_o, lhsT=hT[:, i, k, :], rhs=wo_sb[:, k, ts(c, 512)],
                        start=(k == 0), stop=(k == 3),
                    )
                osb = outp.tile([128, 512], F32, tag="ob", name=f"ob_{i}_{c}")
                nc.vector.tensor_add(out=osb, in0=ps_o, in1=xr_t[:, ts(c, 512)])
                nc.sync.dma_start(out=out_ap[ts(i, 128), ts(c, 512)], in_=osb)

        # ---- phase A: LN + transpose-DMA; V and pair-0 K/Q inline ----
        x_tiles = {}

        def issue_x(i):
            if i < ST and i not in x_tiles:
                x_tiles[i] = xin.tile([128, D], BF, tag="x", name=f"x_{i}")
                nc.sync.dma_start(out=x_tiles[i], in_=x_ap[ts(i, 128), :])

        for i in range(12):
            issue_x(i)
        for i in range(ST):
            x_t = x_tiles.pop(i)
            stats = stat.tile([128, 2, 6], F32, tag="st")
            for sg in range(2):
                nc.vector.bn_stats(out=stats[:, sg, :], in_=x_t[:, ts(sg, 512)])
            mv = stat.tile([128, 2], F32, tag="mv")
            nc.vector.bn_aggr(out=mv, in_=stats)
            std = stat.tile([128, 1], F32, tag="sd")
            nc.scalar.activation(
                out=std, in_=mv[:, 1:2],
                func=mybir.ActivationFunctionType.Sqrt, bias=eps_t,
            )
            istd = stat.tile([128, 1], F32, tag="is")
            nc.vector.reciprocal(out=istd, in_=std)
            xn_t = xnp.tile([128, D], BF, tag="xn")
            nc.vector.tensor_scalar(
                out=xn_t, in0=x_t,
                scalar1=mv[:, 0:1], scalar2=istd,
                op0=mybir.AluOpType.subtract, op1=mybir.AluOpType.mult,
            )
            nc.sync.dma_start_transpose(xnT[:, i], xn_t)
            issue_x(i + 12)
            flush_drains(keep=1)
            if i >= 1:
                emit_v(i - 1)
            if i % 4 == 0 and i > 0:
                emit_qk("k", 0, i // 4 - 1, defer=True)
                emit_qk("k", 1, i // 4 - 1, defer=True)
        emit_v(ST - 1)
        emit_qk("k", 0, NB - 1, defer=True)
        emit_qk("k", 1, NB - 1, defer=True)
        emit_qk("q", 0, 0, defer=True)
        flush_drains()
        flush_v()
        psV_cm.__exit__(None, None, None)
        psS = ctx.enter_context(tc.tile_pool(name="psS", bufs=2, space="PSUM"))
        psB = ctx.enter_context(tc.tile_pool(name="psB", bufs=1, space="PSUM"))

        # remaining projections stream into the Act-bound attention window,
        # ordered by deadline under the blocks-outer schedule: block (n, p)
        # needs K(p, all) and Q(p, n) before it starts.
        work_queue = (
            [("q", 1, 0)]
            + [("k", 2, n) for n in range(NB)] + [("q", 2, 0)]
            + [("k", 3, 0), ("q", 3, 0), ("k", 3, 1), ("k", 3, 2), ("k", 3, 3)]
            + [("q", p, n) for n in range(1, NB) for p in range(NP_)]
        )

        def pop_work():
            if work_queue:
                emit_qk(*work_queue.pop(0))

        if TSPAN == 2:
            pop_slots = (1, 4, 6)
            out_slots = (3, 7)
        else:
            pop_slots = (3, 8, 13)
            out_slots = (6, 15)

        # ---- phase B: attention, blocks outer so each s-block's output
        # completes early and the out-projection streams across the run ----
        blocks = [(p, n) for n in range(NB) for p in range(NP_)]
        ready_out = []
        s12_next = emit_scores(0, 0, 0)
        for bi, (p, n) in enumerate(blocks):
            psv = [
                psB.tile([128, 4, E + 1], F32, tag=f"pv{h}", name=f"pv_{p}_{n}_{h}")
                for h in range(2)
            ]
            for g in range(GT):
                s12 = s12_next
                pt = ptp.tile([128, TSPAN, 2, 512], BF, tag="pt",
                              name=f"pt_{p}_{n}_{g}")
                nc.scalar.activation(
                    out=pt, in_=s12, func=mybir.ActivationFunctionType.Exp
                )
                # prefetch next scores tile so PE stays busy during exp
                if g < GT - 1:
                    s12_next = emit_scores(p, n, g + 1)
                elif bi < len(blocks) - 1:
                    s12_next = emit_scores(*blocks[bi + 1], 0)
                if g in pop_slots:
                    pop_work()
                if g in out_slots and ready_out:
                    emit_out_tile(ready_out.pop(0))
                for tt in range(TSPAN):
                    t = TSPAN * g + tt
                    for h in range(2):
                        hg = 2 * p + h
                        for sc in range(4):
                            # one start/stop per PSUM bank: the sc==0 start
                            # zeroes the whole 2KB region for all 4 slots
                            nc.tensor.matmul(
                                psv[h][:, sc, :],
                                lhsT=pt[:, tt, h, ts(sc, 128)],
                                rhs=v_sb[:, t, hg, :],
                                start=(t == 0 and sc == 0),
                                stop=(t == TT - 1 and sc == 3),
                                skip_group_check=True,
                            )
            # block epilogue: normalize into hC (denominator is psv[..., E])
            r8 = r8p.tile([128, HL], F32, tag="r8", name=f"r8_{p}_{n}")
            for h in range(2):
                nc.vector.reciprocal(out=r8[:, 4 * h : 4 * h + 4], in_=psv[h][:, :, E])
            for j in range(HL):
                h, sc = j // 4, j % 4
                nc.vector.tensor_scalar_mul(
                    out=hC[:, n * 4 + sc, ts(2 * p + h, E)],
                    in0=psv[h][:, sc, 0:E],
                    scalar1=r8[:, j : j + 1],
                )
            if p == NP_ - 1:
                for sc in range(4):
                    i = n * 4 + sc
                    nc.sync.dma_start_transpose(hT[:, i], hC[:, i, :])
                ready_out.extend(range(n * 4, n * 4 + 4))
        # remaining out-projection tiles
        while ready_out:
            emit_out_tile(ready_out.pop(0))

        aps["_dbg"] = {
            "xnT": xnT.tensor.name, "qT": qT.tensor.name, "kT_": kT_.tensor.name,
            "v_sb": v_sb.tensor.name, "hC": hC.tensor.name, "hT": hT.tensor.name,
        }


def build():
    nc = bacc.Bacc("TRN2", target_bir_lowering=False, debug=False, num_devices=N_CORES)
    aps = {
        "x": nc.dram_tensor("x", [S, D], BF, kind="ExternalInput").ap(),
        "xr": nc.dram_tensor("xr", [S, D], F32, kind="ExternalInput").ap(),
        "wq": nc.dram_tensor("wq", [KT, 128, 512], BF, kind="ExternalInput").ap(),
        "wk": nc.dram_tensor("wk", [KT, 128, 512], BF, kind="ExternalInput").ap(),
        "wv": nc.dram_tensor("wv", [KT, 128, 512], BF, kind="ExternalInput").ap(),
        "wo": nc.dram_tensor("wo", [4, 128, 1024], BF, kind="ExternalInput").ap(),
        "bq": nc.dram_tensor("bq", [128, NP_], F32, kind="ExternalInput").ap(),
        "bk": nc.dram_tensor("bk", [128, NP_], F32, kind="ExternalInput").ap(),
        "out": nc.dram_tensor("out", [S, D], F32, kind="ExternalOutput").ap(),
    }
    with tile.TileContext(nc) as tc:
        aps["tc"] = tc
        _emit(nc, aps)
    nc._dbg_names = aps.get("_dbg", {})
    nc.compile()
    return nc


def prep_core_inputs(x, Wq, bq, Wk, bk, Wv, bv, Wo, bo, ln_gamma, ln_beta):
    """Host-side sharding: returns list of 8 in_maps (numpy arrays)."""
    x = np.asarray(x, np.float32)
    Wq, bq = np.asarray(Wq, np.float32), np.asarray(bq, np.float32)
    Wk, bk = np.asarray(Wk, np.float32), np.asarray(bk, np.float32)
    Wv, bv = np.asarray(Wv, np.float32), np.asarray(bv, np.float32)
    Wo, bo = np.asarray(Wo, np.float32), np.asarray(bo, np.float32)
    gamma, beta = np.asarray(ln_gamma, np.float32), np.asarray(ln_beta, np.float32)

    Wq_eff = Wq * gamma[None, None, :] / SCALE
    bq_eff = (bq + Wq @ beta) / SCALE
    Wk_eff = Wk * gamma[None, None, :]
    bk_eff = bk + Wk @ beta
    Wv_eff = Wv * gamma[None, None, :]
    bv_eff = bv + Wv @ beta

    def wq_layout(w):  # [8, 64, 1024] -> [KT, 128, 512]
        # w[h, e, kt*128+dd] -> out[kt, dd, h*64+e]
        return np.ascontiguousarray(
            w.reshape(HL * E, KT, 128).transpose(1, 2, 0)
        ).astype(BF_NP)

    def b_layout(b):  # [8, 64] -> [128, 4]: out[(hh*64+e), p] = b[2p+hh, e]
        return np.ascontiguousarray(
            b.reshape(NP_, 2 * E).T
        ).astype(np.float32)

    in_maps = []
    for c in range(N_CORES):
        bidx, g = c // 2, c % 2
        hs = slice(g * HL, (g + 1) * HL)
        wo_loc = Wo[:, g * 512:(g + 1) * 512]  # [1024, 512]
        wo_dev = np.ascontiguousarray(
            wo_loc.T.reshape(4, 128, 1024)
        ).astype(BF_NP)
        bv_l = bv_eff[hs].reshape(512)
        in_maps.append({
            "x": x[bidx].astype(BF_NP),
            "xr": 0.5 * (x[bidx] + bo[None, :]) + (wo_loc.astype(np.float32) @ bv_l)[None, :],
            "wq": wq_layout(Wq_eff[hs]),
            "wk": wq_layout(Wk_eff[hs]),
            "wv": wq_layout(Wv_eff[hs]),
            "wo": wo_dev,
            "bq": b_layout(bq_eff[hs]),
            "bk": b_layout(bk_eff[hs]),
            "out": np.zeros((S, D), np.float32),
        })
    return in_maps


def kernel(x, Wq, bq, Wk, bk, Wv, bv, Wo, bo, ln_gamma, ln_beta):
    global _NC_CACHE
    if _NC_CACHE is None:
        _NC_CACHE = build()
    nc = _NC_CACHE
    in_maps = prep_core_inputs(x, Wq, bq, Wk, bk, Wv, bv, Wo, bo, ln_gamma, ln_beta)
    for m in in_maps:
        m.pop("out")
    res = bass_utils.run_bass_kernel_spmd(nc, in_maps, core_ids=list(range(N_CORES)))
    out = np.empty((B, S, D), np.float32)
    for bidx in range(B):
        out[bidx] = res.results[2 * bidx]["out"] + res.results[2 * bidx + 1]["out"]
    return out


# revision 24
# speedup vs baseline: 1.1584x; 1.0577x over previous
"""Multi-head attention block (pre-LN, residual) on 8 Trainium2 NeuronCores.

Sharding: (batch x head-group) grid. Core c handles batch b = c//2 and head
group g = c%2 (8 of 16 heads). Per core: LN(x_b) -> per-head QKV projections
-> softmax attention (no max-subtraction; scores are O(10)) -> out-projection
against the local 512-wide slice of Wo, + 0.5*(x+bo) residual. Host sums the
two partial outputs per batch (the pair all-reduce) and stacks batches.

v2 structure:
- LN apply moved to the Act engine (Identity activation with per-partition
  scale/bias), stats stay on DVE.
- xn -> xnT via DMA crossbar transposes (one per s-tile) instead of PE
  identity-matmul transposes + vector copies.
- PV matmuls flipped to [s, e] output orientation (free size 65 instead of
  512) with a ones-column appended to V so the softmax denominator falls out
  of the PE accumulation for free; normalization is folded into the
  PSUM->SBUF copy. Removes all denominator adds and broadcast-back matmuls.
- Attention heads are concatenated in [s, he] layout (hC) and DMA-crossbar
  transposed per s-tile into hT for the out-projection.
- Scores accumulate in bf16 PSUM so an exp instruction spans 2 t-tiles
  (2048 elements), halving Act-engine instruction overhead.

LayerNorm gamma/beta are folded into the QKV weights/biases on the host
(exact: projections are linear in xn). The 1/sqrt(E) score scale is folded
into Wq. Matmul operands are bf16 with fp32 PSUM accumulation for
projections/PV/out-proj; LN statistics and the residual path stay fp32.
"""

import numpy as np
import ml_dtypes

import concourse.bass as bass
import concourse.mybir as mybir
import concourse.tile as tile
from concourse import bacc
from concourse import bass_utils
from concourse.bass import ts

BF_NP = ml_dtypes.bfloat16

B, S, D = 4, 2048, 1024
H, E = 16, 64
LN_EPS = 1e-5
SCALE = 8.0  # sqrt(E) * TEMP

N_CORES = 8
HL = H // 2          # heads per core
NP_ = HL // 2        # 4 head pairs per core
KT = D // 128        # 8 contraction tiles for D
ST = S // 128        # 16 s-tiles of 128
NB = S // 512        # 4 s-blocks of 512
TT = S // 128        # 16 t-tiles of 128

F32 = mybir.dt.float32
BF = mybir.dt.bfloat16

# Score PSUM dtype: bf16 lets one exp instruction span TSPAN=2 t-tiles within
# the PSUM budget. Fallback (if bf16 matmul-out to PSUM misbehaves):
# F32S=True -> f32 scores, TSPAN=1.
F32S = True
TSPAN = 1 if F32S else 2
GT = TT // TSPAN     # groups per block
SDT = F32 if F32S else BF

_NC_CACHE = None


def _emit(nc, aps):
    x_ap = aps["x"]
    xr_ap = aps["xr"]
    wq_ap, wk_ap, wv_ap, wo_ap = aps["wq"], aps["wk"], aps["wv"], aps["wo"]
    bq_ap, bk_ap = aps["bq"], aps["bk"]
    out_ap = aps["out"]

    tc = aps["tc"]
    import contextlib

    ctx = contextlib.ExitStack()
    with ctx:
        const = ctx.enter_context(tc.tile_pool(name="const", bufs=1))
        big = ctx.enter_context(tc.tile_pool(name="big", bufs=1))
        xin = ctx.enter_context(tc.tile_pool(name="xin", bufs=3))
        stat = ctx.enter_context(tc.tile_pool(name="stat", bufs=8))
        xnp = ctx.enter_context(tc.tile_pool(name="xnp", bufs=6))
        ptp = ctx.enter_context(tc.tile_pool(name="ptp", bufs=3))
        r8p = ctx.enter_context(tc.tile_pool(name="r8p", bufs=2))
        xrp = ctx.enter_context(tc.tile_pool(name="xrp", bufs=2))
        outp = ctx.enter_context(tc.tile_pool(name="outp", bufs=2))
        psA = ctx.enter_context(tc.tile_pool(name="psA", bufs=2, space="PSUM"))
        # psV is phase-A scoped: its 4 banks are released before the
        # attention pools (psS/psB) are created, so V-projection drains can
        # lag 3 tiles behind their matmuls without blocking the ring.
        psV_cm = tc.tile_pool(name="psV", bufs=4, space="PSUM")
        psV = psV_cm.__enter__()

        # ---- constants / weights resident in SBUF ----
        # wv/wk ride the fast HWDGE (sync) queue right after the first x
        # tiles: V(0) needs wv at ~7us, K(0,0) needs wk at ~10us. The SWDGE
        # (gpsimd) path costs ~1us Pool-engine time per DMA, which starved
        # the projections when all weights went through it.
        wq_sb = const.tile([128, KT, 512], BF, tag="wq")
        wk_sb = const.tile([128, KT, 512], BF, tag="wk")
        wv_sb = const.tile([128, KT, 512], BF, tag="wv")
        wo_sb = const.tile([128, 4, 1024], BF, tag="wo")
        bq_sb = const.tile([128, NP_], F32, tag="bq")
        bk_sb = const.tile([128, NP_], F32, tag="bk")
        # all weights ride the gpsimd SWDGE queue (Pool engine is otherwise
        # idle), most-urgent first: wv for V(0) at ~9us, wk for K(p0),
        # wq/wo much later. Keeps both HWDGE queues free for x tiles and
        # transposes. bv is folded into xr on the host (exact: the softmax
        # rows sum to one, so +bv passes through attention untouched).
        nc.gpsimd.dma_start(out=bq_sb, in_=bq_ap)
        nc.gpsimd.dma_start(out=bk_sb, in_=bk_ap)

        def w_ap(src, kt, free):
            return bass.AP(tensor=src.tensor, offset=src.offset,
                           ap=[[free, 128], [128 * free, kt], [1, free]])

        nc.gpsimd.dma_start(out=wv_sb, in_=w_ap(wv_ap, KT, 512))
        nc.gpsimd.dma_start(out=wk_sb, in_=w_ap(wk_ap, KT, 512))
        nc.gpsimd.dma_start(out=wq_sb, in_=w_ap(wq_ap, KT, 512))
        nc.gpsimd.dma_start(out=wo_sb, in_=w_ap(wo_ap, 4, 1024))
        eps_t = const.tile([128, 1], F32, tag="eps")
        nc.vector.memset(eps_t, LN_EPS)

        # [d%128, s_tile, d//128, s%128] LN(x) transposed, per-tile contiguous
        xnT = big.tile([128, ST, KT, 128], BF, tag="xnT")
        qT = big.tile([128, NP_, S], BF, tag="qT")      # [(pairhead,e), s]
        kT_ = big.tile([128, NP_, S], BF, tag="kT")
        v_sb = big.tile([128, TT, HL, E + 1], BF, tag="v")  # [t, h, e | ones]
        hC = big.tile([128, ST, 512], BF, tag="hC")     # [s, (h,e)] attn out
        hT = big.tile([128, ST, 4, 128], BF, tag="hT")  # [(he)%128, s_tile, (he)//128, s%128]
        # ones column of V' (denominator accumulator feed)
        nc.gpsimd.memset(v_sb[:, :, :, E : E + 1], 1.0)

        # ---- work units ----
        # Phase-A PSUM drains are deferred to the next tile so no in-order
        # DVE instruction ever waits on the transpose->matmul chain (a DVE
        # stall there blocks the next tile's LN stats behind it).
        drains = []

        def flush_drains(keep=0):
            while len(drains) > keep:
                drains.pop(0)()

        def emit_qk(kind, p, n, defer=False):
            # never leave >1 undrained psA unit: the ring has 2 slots and a
            # writer emitted before the previous reader is a race
            flush_drains(keep=1)
            w_sb, b_sb, dst = (
                (wq_sb, bq_sb, qT) if kind == "q" else (wk_sb, bk_sb, kT_)
            )
            ps = psA.tile([128, 512], F32, tag="ps", name=f"proj_{kind}_{p}_{n}")
            for k in range(KT):
                nc.tensor.matmul(
                    ps, lhsT=w_sb[:, k, ts(p, 128)],
                    rhs=xnT[:, 4 * n : 4 * n + 4, k, :],
                    start=(k == 0), stop=(k == KT - 1),
                )

            def drain_act():
                # per-partition bias add on the Act engine (phase A: DVE is
                # saturated by the LN chain, Act is idle)
                nc.scalar.activation(
                    out=dst[:, p, ts(n, 512)], in_=ps,
                    func=mybir.ActivationFunctionType.Identity,
                    bias=b_sb[:, p : p + 1],
                )

            def drain_dve():
                nc.vector.tensor_scalar_add(
                    out=dst[:, p, ts(n, 512)], in0=ps, scalar1=b_sb[:, p : p + 1]
                )

            if defer:
                drains.append(drain_act)
            else:
                drain_dve()

        v_drains = []

        def flush_v(keep=0):
            while len(v_drains) > keep:
                v_drains.pop(0)()

        def emit_v(t):
            flush_v(keep=3)
            ps = psV.tile([128, HL, E], F32, tag="psv", name=f"proj_v_{t}")
            for k in range(KT):
                nc.tensor.matmul(
                    ps, lhsT=xnT[:, t, k, :], rhs=wv_sb[:, k, :],
                    start=(k == 0), stop=(k == KT - 1),
                )
            v_drains.append(
                lambda: nc.scalar.copy(out=v_sb[:, t, :, 0:E], in_=ps)
            )

        def emit_scores(p, n, g):
            s12 = psS.tile([128, TSPAN, 2, 512], SDT, tag="s12",
                           name=f"s12_{p}_{n}_{g}")
            for tt in range(TSPAN):
                t = TSPAN * g + tt
                nc.tensor.matmul(
                    s12[:, tt, 0, :], lhsT=kT_[0:64, p, ts(t, 128)],
                    rhs=qT[0:64, p, ts(n, 512)],
                    start=True, stop=True, tile_position=(0, 0),
                )
                nc.tensor.matmul(
                    s12[:, tt, 1, :], lhsT=kT_[64:128, p, ts(t, 128)],
                    rhs=qT[64:128, p, ts(n, 512)],
                    start=True, stop=True, tile_position=(64, 0),
                )
            return s12

        def emit_out_tile(i):
            xr_t = xrp.tile([128, D], F32, tag="xr", name=f"xr_{i}")
            nc.gpsimd.dma_start(out=xr_t, in_=xr_ap[ts(i, 128), :])
            osb = outp.tile([128, D], F32, tag="ob", name=f"ob_{i}")
            for c in range(2):
                ps_o = psA.tile([128, 512], F32, tag="ps", name=f"pso_{i}_{c}")
                for k in range(4):
                    nc.tensor.matmul(
                        ps_o, lhsT=hT[:, i, k, :], rhs=wo_sb[:, k, ts(c, 512)],
                        start=(k == 0), stop=(k == 3),
                    )
                nc.vector.tensor_add(
                    out=osb[:, ts(c, 512)], in0=ps_o, in1=xr_t[:, ts(c, 512)]
                )
            nc.sync.dma_start(out=out_ap[ts(i, 128), :], in_=osb)

        # ---- phase A: LN + transpose-DMA; V and pair-0 K/Q inline ----
        x_groups = {}

        def issue_xg(g):
            if g < ST // 4 and g not in x_groups:
                x_groups[g] = xin.tile([128, 4, D], BF, tag="x", name=f"xg_{g}")
                src_ap = bass.AP(
                    tensor=x_ap.tensor,
                    offset=x_ap.offset + g * 512 * D,
                    ap=[[D, 128], [128 * D, 4], [1, D]],
                )
                nc.sync.dma_start(out=x_groups[g], in_=src_ap)

        for g in range(3):
            issue_xg(g)
        for i in range(ST):
            if i == 4:
                issue_xg(3)
            x_t = x_groups[i // 4][:, i % 4, :]
            stats = stat.tile([128, 2, 6], F32, tag="st")
            for sg in range(2):
                nc.vector.bn_stats(out=stats[:, sg, :], in_=x_t[:, ts(sg, 512)])
            mv = stat.tile([128, 2], F32, tag="mv")
            nc.vector.bn_aggr(out=mv, in_=stats)
            std = stat.tile([128, 1], F32, tag="sd")
            nc.scalar.activation(
                out=std, in_=mv[:, 1:2],
                func=mybir.ActivationFunctionType.Sqrt, bias=eps_t,
            )
            istd = stat.tile([128, 1], F32, tag="is")
            nc.vector.reciprocal(out=istd, in_=std)
            xn_t = xnp.tile([128, D], BF, tag="xn")
            nc.vector.tensor_scalar(
                out=xn_t, in0=x_t,
                scalar1=mv[:, 0:1], scalar2=istd,
                op0=mybir.AluOpType.subtract, op1=mybir.AluOpType.mult,
            )
            nc.sync.dma_start_transpose(xnT[:, i], xn_t)
            flush_drains(keep=1)
            if i >= 1:
                emit_v(i - 1)
            if i % 4 == 0 and i > 0:
                emit_qk("k", 0, i // 4 - 1, defer=True)
                emit_qk("k", 1, i // 4 - 1, defer=True)
        emit_v(ST - 1)
        emit_qk("k", 0, NB - 1, defer=True)
        emit_qk("k", 1, NB - 1, defer=True)
        emit_qk("q", 0, 0, defer=True)
        flush_drains()
        flush_v()
        psV_cm.__exit__(None, None, None)
        psS = ctx.enter_context(tc.tile_pool(name="psS", bufs=2, space="PSUM"))
        psB = ctx.enter_context(tc.tile_pool(name="psB", bufs=1, space="PSUM"))

        # remaining projections stream into the Act-bound attention window,
        # ordered by deadline under the blocks-outer schedule: block (n, p)
        # needs K(p, all) and Q(p, n) before it starts.
        work_queue = (
            [("q", 1, 0)]
            + [("k", 2, n) for n in range(NB)] + [("q", 2, 0)]
            + [("k", 3, 0), ("q", 3, 0), ("k", 3, 1), ("k", 3, 2), ("k", 3, 3)]
            + [("q", p, n) for n in range(1, NB) for p in range(NP_)]
        )

        def pop_work():
            if work_queue:
                emit_qk(*work_queue.pop(0))

        if TSPAN == 2:
            pop_slots = (1, 4, 6)
            out_slots = (3, 7)
        else:
            pop_slots = (3, 8, 13)
            out_slots = (6, 15)

        # ---- phase B: attention, blocks outer so each s-block's output
        # completes early and the out-projection streams across the run ----
        blocks = [(p, n) for n in range(NB) for p in range(NP_)]
        ready_out = []
        s12_next = emit_scores(0, 0, 0)
        for bi, (p, n) in enumerate(blocks):
            psv = [
                psB.tile([128, 4, E + 1], F32, tag=f"pv{h}", name=f"pv_{p}_{n}_{h}")
                for h in range(2)
            ]
            for g in range(GT):
                s12 = s12_next
                pt = ptp.tile([128, TSPAN, 2, 512], BF, tag="pt",
                              name=f"pt_{p}_{n}_{g}")
                nc.scalar.activation(
                    out=pt, in_=s12, func=mybir.ActivationFunctionType.Exp
                )
                # prefetch next scores tile so PE stays busy during exp
                if g < GT - 1:
                    s12_next = emit_scores(p, n, g + 1)
                elif bi < len(blocks) - 1:
                    s12_next = emit_scores(*blocks[bi + 1], 0)
                if g in pop_slots:
                    pop_work()
                if g in out_slots and ready_out:
                    emit_out_tile(ready_out.pop(0))
                for tt in range(TSPAN):
                    t = TSPAN * g + tt
                    for h in range(2):
                        hg = 2 * p + h
                        for sc in range(4):
                            # one start/stop per PSUM bank: the sc==0 start
                            # zeroes the whole 2KB region for all 4 slots
                            nc.tensor.matmul(
                                psv[h][:, sc, :],
                                lhsT=pt[:, tt, h, ts(sc, 128)],
                                rhs=v_sb[:, t, hg, :],
                                start=(t == 0 and sc == 0),
                                stop=(t == TT - 1 and sc == 3),
                                skip_group_check=True,
                            )
            # block epilogue: normalize into hC (denominator is psv[..., E])
            r8 = r8p.tile([128, HL], F32, tag="r8", name=f"r8_{p}_{n}")
            for h in range(2):
                nc.vector.reciprocal(out=r8[:, 4 * h : 4 * h + 4], in_=psv[h][:, :, E])
            for j in range(HL):
                h, sc = j // 4, j % 4
                nc.vector.tensor_scalar_mul(
                    out=hC[:, n * 4 + sc, ts(2 * p + h, E)],
                    in0=psv[h][:, sc, 0:E],
                    scalar1=r8[:, j : j + 1],
                )
            if p == NP_ - 1:
                for sc in range(4):
                    i = n * 4 + sc
                    nc.sync.dma_start_transpose(hT[:, i], hC[:, i, :])
                ready_out.extend(range(n * 4, n * 4 + 4))
        # remaining out-projection tiles
        while ready_out:
            emit_out_tile(ready_out.pop(0))

        aps["_dbg"] = {
            "xnT": xnT.tensor.name, "qT": qT.tensor.name, "kT_": kT_.tensor.name,
            "v_sb": v_sb.tensor.name, "hC": hC.tensor.name, "hT": hT.tensor.name,
        }


def build():
    nc = bacc.Bacc("TRN2", target_bir_lowering=False, debug=False, num_devices=N_CORES)
    aps = {
        "x": nc.dram_tensor("x", [S, D], BF, kind="ExternalInput").ap(),
        "xr": nc.dram_tensor("xr", [S, D], F32, kind="ExternalInput").ap(),
        "wq": nc.dram_tensor("wq", [KT, 128, 512], BF, kind="ExternalInput").ap(),
        "wk": nc.dram_tensor("wk", [KT, 128, 512], BF, kind="ExternalInput").ap(),
        "wv": nc.dram_tensor("wv", [KT, 128, 512], BF, kind="ExternalInput").ap(),
        "wo": nc.dram_tensor("wo", [4, 128, 1024], BF, kind="ExternalInput").ap(),
        "bq": nc.dram_tensor("bq", [128, NP_], F32, kind="ExternalInput").ap(),
        "bk": nc.dram_tensor("bk", [128, NP_], F32, kind="ExternalInput").ap(),
        "out": nc.dram_tensor("out", [S, D], F32, kind="ExternalOutput").ap(),
    }
    with tile.TileContext(nc) as tc:
        aps["tc"] = tc
        _emit(nc, aps)
    nc._dbg_names = aps.get("_dbg", {})
    nc.compile()
    return nc


def prep_core_inputs(x, Wq, bq, Wk, bk, Wv, bv, Wo, bo, ln_gamma, ln_beta):
    """Host-side sharding: returns list of 8 in_maps (numpy arrays)."""
    x = np.asarray(x, np.float32)
    Wq, bq = np.asarray(Wq, np.float32), np.asarray(bq, np.float32)
    Wk, bk = np.asarray(Wk, np.float32), np.asarray(bk, np.float32)
    Wv, bv = np.asarray(Wv, np.float32), np.asarray(bv, np.float32)
    Wo, bo = np.asarray(Wo, np.float32), np.asarray(bo, np.float32)
    gamma, beta = np.asarray(ln_gamma, np.float32), np.asarray(ln_beta, np.float32)

    Wq_eff = Wq * gamma[None, None, :] / SCALE
    bq_eff = (bq + Wq @ beta) / SCALE
    Wk_eff = Wk * gamma[None, None, :]
    bk_eff = bk + Wk @ beta
    Wv_eff = Wv * gamma[None, None, :]
    bv_eff = bv + Wv @ beta

    def wq_layout(w):  # [8, 64, 1024] -> [KT, 128, 512]
        # w[h, e, kt*128+dd] -> out[kt, dd, h*64+e]
        return np.ascontiguousarray(
            w.reshape(HL * E, KT, 128).transpose(1, 2, 0)
        ).astype(BF_NP)

    def b_layout(b):  # [8, 64] -> [128, 4]: out[(hh*64+e), p] = b[2p+hh, e]
        return np.ascontiguousarray(
            b.reshape(NP_, 2 * E).T
        ).astype(np.float32)

    in_maps = []
    for c in range(N_CORES):
        bidx, g = c // 2, c % 2
        hs = slice(g * HL, (g + 1) * HL)
        wo_loc = Wo[:, g * 512:(g + 1) * 512]  # [1024, 512]
        wo_dev = np.ascontiguousarray(
            wo_loc.T.reshape(4, 128, 1024)
        ).astype(BF_NP)
        bv_l = bv_eff[hs].reshape(512)
        in_maps.append({
            "x": x[bidx].astype(BF_NP),
            "xr": 0.5 * (x[bidx] + bo[None, :]) + (wo_loc.astype(np.float32) @ bv_l)[None, :],
            "wq": wq_layout(Wq_eff[hs]),
            "wk": wq_layout(Wk_eff[hs]),
            "wv": wq_layout(Wv_eff[hs]),
            "wo": wo_dev,
            "bq": b_layout(bq_eff[hs]),
            "bk": b_layout(bk_eff[hs]),
            "out": np.zeros((S, D), np.float32),
        })
    return in_maps


def kernel(x, Wq, bq, Wk, bk, Wv, bv, Wo, bo, ln_gamma, ln_beta):
    global _NC_CACHE
    if _NC_CACHE is None:
        _NC_CACHE = build()
    nc = _NC_CACHE
    in_maps = prep_core_inputs(x, Wq, bq, Wk, bk, Wv, bv, Wo, bo, ln_gamma, ln_beta)
    for m in in_maps:
        m.pop("out")
    res = bass_utils.run_bass_kernel_spmd(nc, in_maps, core_ids=list(range(N_CORES)))
    out = np.empty((B, S, D), np.float32)
    for bidx in range(B):
        out[bidx] = res.results[2 * bidx]["out"] + res.results[2 * bidx + 1]["out"]
    return out


# revision 25
# speedup vs baseline: 1.1675x; 1.0079x over previous
"""Multi-head attention block (pre-LN, residual) on 8 Trainium2 NeuronCores.

Sharding: (batch x head-group) grid. Core c handles batch b = c//2 and head
group g = c%2 (8 of 16 heads). Per core: LN(x_b) -> per-head QKV projections
-> softmax attention (no max-subtraction; scores are O(10)) -> out-projection
against the local 512-wide slice of Wo, + 0.5*(x+bo) residual. Host sums the
two partial outputs per batch (the pair all-reduce) and stacks batches.

v2 structure:
- LN apply moved to the Act engine (Identity activation with per-partition
  scale/bias), stats stay on DVE.
- xn -> xnT via DMA crossbar transposes (one per s-tile) instead of PE
  identity-matmul transposes + vector copies.
- PV matmuls flipped to [s, e] output orientation (free size 65 instead of
  512) with a ones-column appended to V so the softmax denominator falls out
  of the PE accumulation for free; normalization is folded into the
  PSUM->SBUF copy. Removes all denominator adds and broadcast-back matmuls.
- Attention heads are concatenated in [s, he] layout (hC) and DMA-crossbar
  transposed per s-tile into hT for the out-projection.
- Scores accumulate in bf16 PSUM so an exp instruction spans 2 t-tiles
  (2048 elements), halving Act-engine instruction overhead.

LayerNorm gamma/beta are folded into the QKV weights/biases on the host
(exact: projections are linear in xn). The 1/sqrt(E) score scale is folded
into Wq. Matmul operands are bf16 with fp32 PSUM accumulation for
projections/PV/out-proj; LN statistics and the residual path stay fp32.
"""

import numpy as np
import ml_dtypes

import concourse.bass as bass
import concourse.mybir as mybir
import concourse.tile as tile
from concourse import bacc
from concourse import bass_utils
from concourse.bass import ts

BF_NP = ml_dtypes.bfloat16

B, S, D = 4, 2048, 1024
H, E = 16, 64
LN_EPS = 1e-5
SCALE = 8.0  # sqrt(E) * TEMP

N_CORES = 8
HL = H // 2          # heads per core
NP_ = HL // 2        # 4 head pairs per core
KT = D // 128        # 8 contraction tiles for D
ST = S // 128        # 16 s-tiles of 128
NB = S // 512        # 4 s-blocks of 512
TT = S // 128        # 16 t-tiles of 128

F32 = mybir.dt.float32
BF = mybir.dt.bfloat16

# Score PSUM dtype: bf16 lets one exp instruction span TSPAN=2 t-tiles within
# the PSUM budget. Fallback (if bf16 matmul-out to PSUM misbehaves):
# F32S=True -> f32 scores, TSPAN=1.
F32S = True
TSPAN = 1 if F32S else 2
GT = TT // TSPAN     # groups per block
SDT = F32 if F32S else BF

_NC_CACHE = None


def _emit(nc, aps):
    x_ap = aps["x"]
    xr_ap = aps["xr"]
    wq_ap, wk_ap, wv_ap, wo_ap = aps["wq"], aps["wk"], aps["wv"], aps["wo"]
    bq_ap, bk_ap = aps["bq"], aps["bk"]
    out_ap = aps["out"]

    tc = aps["tc"]
    import contextlib

    ctx = contextlib.ExitStack()
    with ctx:
        const = ctx.enter_context(tc.tile_pool(name="const", bufs=1))
        big = ctx.enter_context(tc.tile_pool(name="big", bufs=1))
        xin = ctx.enter_context(tc.tile_pool(name="xin", bufs=3))
        stat = ctx.enter_context(tc.tile_pool(name="stat", bufs=8))
        xnp = ctx.enter_context(tc.tile_pool(name="xnp", bufs=6))
        ptp = ctx.enter_context(tc.tile_pool(name="ptp", bufs=3))
        r8p = ctx.enter_context(tc.tile_pool(name="r8p", bufs=2))
        xrp = ctx.enter_context(tc.tile_pool(name="xrp", bufs=2))
        outp = ctx.enter_context(tc.tile_pool(name="outp", bufs=2))
        psA = ctx.enter_context(tc.tile_pool(name="psA", bufs=2, space="PSUM"))
        # psV is phase-A scoped: its 4 banks are released before the
        # attention pools (psS/psB) are created, so V-projection drains can
        # lag 3 tiles behind their matmuls without blocking the ring.
        psV_cm = tc.tile_pool(name="psV", bufs=4, space="PSUM")
        psV = psV_cm.__enter__()

        # ---- constants / weights resident in SBUF ----
        # wv/wk ride the fast HWDGE (sync) queue right after the first x
        # tiles: V(0) needs wv at ~7us, K(0,0) needs wk at ~10us. The SWDGE
        # (gpsimd) path costs ~1us Pool-engine time per DMA, which starved
        # the projections when all weights went through it.
        wq_sb = const.tile([128, KT, 512], BF, tag="wq")
        wk_sb = const.tile([128, KT, 512], BF, tag="wk")
        wv_sb = const.tile([128, KT, 512], BF, tag="wv")
        wo_sb = const.tile([128, 4, 1024], BF, tag="wo")
        bq_sb = const.tile([128, NP_], F32, tag="bq")
        bk_sb = const.tile([128, NP_], F32, tag="bk")
        # all weights ride the gpsimd SWDGE queue (Pool engine is otherwise
        # idle), most-urgent first: wv for V(0) at ~9us, wk for K(p0),
        # wq/wo much later. Keeps both HWDGE queues free for x tiles and
        # transposes. bv is folded into xr on the host (exact: the softmax
        # rows sum to one, so +bv passes through attention untouched).
        nc.gpsimd.dma_start(out=bq_sb, in_=bq_ap)
        nc.gpsimd.dma_start(out=bk_sb, in_=bk_ap)

        def w_ap(src, kt, free):
            return bass.AP(tensor=src.tensor, offset=src.offset,
                           ap=[[free, 128], [128 * free, kt], [1, free]])

        nc.gpsimd.dma_start(out=wv_sb, in_=w_ap(wv_ap, KT, 512))
        nc.gpsimd.dma_start(out=wk_sb, in_=w_ap(wk_ap, KT, 512))
        nc.gpsimd.dma_start(out=wq_sb, in_=w_ap(wq_ap, KT, 512))
        nc.gpsimd.dma_start(out=wo_sb, in_=w_ap(wo_ap, 4, 1024))
        eps_t = const.tile([128, 1], F32, tag="eps")
        nc.vector.memset(eps_t, LN_EPS)

        # [d%128, s_tile, d//128, s%128] LN(x) transposed, per-tile contiguous
        xnT = big.tile([128, ST, KT, 128], BF, tag="xnT")
        qT = big.tile([128, NP_, S], BF, tag="qT")      # [(pairhead,e), s]
        kT_ = big.tile([128, NP_, S], BF, tag="kT")
        v_sb = big.tile([128, TT, HL, E + 1], BF, tag="v")  # [t, h, e | ones]
        hC = big.tile([128, ST, 512], BF, tag="hC")     # [s, (h,e)] attn out
        hT = big.tile([128, ST, 4, 128], BF, tag="hT")  # [(he)%128, s_tile, (he)//128, s%128]
        # ones column of V' (denominator accumulator feed)
        nc.gpsimd.memset(v_sb[:, :, :, E : E + 1], 1.0)

        # ---- work units ----
        # Phase-A PSUM drains are deferred to the next tile so no in-order
        # DVE instruction ever waits on the transpose->matmul chain (a DVE
        # stall there blocks the next tile's LN stats behind it).
        drains = []

        def flush_drains(keep=0):
            while len(drains) > keep:
                drains.pop(0)()

        def emit_qk(kind, p, n, defer=False):
            # never leave >1 undrained psA unit: the ring has 2 slots and a
            # writer emitted before the previous reader is a race
            flush_drains(keep=1)
            w_sb, b_sb, dst = (
                (wq_sb, bq_sb, qT) if kind == "q" else (wk_sb, bk_sb, kT_)
            )
            ps = psA.tile([128, 512], F32, tag="ps", name=f"proj_{kind}_{p}_{n}")
            for k in range(KT):
                nc.tensor.matmul(
                    ps, lhsT=w_sb[:, k, ts(p, 128)],
                    rhs=xnT[:, 4 * n : 4 * n + 4, k, :],
                    start=(k == 0), stop=(k == KT - 1),
                )

            def drain_act():
                # per-partition bias add on the Act engine (phase A: DVE is
                # saturated by the LN chain, Act is idle)
                nc.scalar.activation(
                    out=dst[:, p, ts(n, 512)], in_=ps,
                    func=mybir.ActivationFunctionType.Identity,
                    bias=b_sb[:, p : p + 1],
                )

            def drain_dve():
                nc.vector.tensor_scalar_add(
                    out=dst[:, p, ts(n, 512)], in0=ps, scalar1=b_sb[:, p : p + 1]
                )

            if defer:
                drains.append(drain_act)
            else:
                drain_dve()

        v_drains = []

        def flush_v(keep=0):
            while len(v_drains) > keep:
                v_drains.pop(0)()

        def emit_v(t, streamed=False):
            if streamed:
                ps = psA.tile([128, HL, E], F32, tag="ps", name=f"proj_v_{t}")
            else:
                flush_v(keep=3)
                ps = psV.tile([128, HL, E], F32, tag="psv", name=f"proj_v_{t}")
            for k in range(KT):
                nc.tensor.matmul(
                    ps, lhsT=xnT[:, t, k, :], rhs=wv_sb[:, k, :],
                    start=(k == 0), stop=(k == KT - 1),
                )
            if streamed:
                nc.vector.tensor_copy(out=v_sb[:, t, :, 0:E], in_=ps)
            else:
                v_drains.append(
                    lambda: nc.scalar.copy(out=v_sb[:, t, :, 0:E], in_=ps)
                )

        def emit_scores(p, n, g):
            s12 = psS.tile([128, TSPAN, 2, 512], SDT, tag="s12",
                           name=f"s12_{p}_{n}_{g}")
            for tt in range(TSPAN):
                t = TSPAN * g + tt
                nc.tensor.matmul(
                    s12[:, tt, 0, :], lhsT=kT_[0:64, p, ts(t, 128)],
                    rhs=qT[0:64, p, ts(n, 512)],
                    start=True, stop=True, tile_position=(0, 0),
                )
                nc.tensor.matmul(
                    s12[:, tt, 1, :], lhsT=kT_[64:128, p, ts(t, 128)],
                    rhs=qT[64:128, p, ts(n, 512)],
                    start=True, stop=True, tile_position=(64, 0),
                )
            return s12

        def emit_out_tile(i):
            xr_t = xrp.tile([128, D], F32, tag="xr", name=f"xr_{i}")
            nc.gpsimd.dma_start(out=xr_t, in_=xr_ap[ts(i, 128), :])
            osb = outp.tile([128, D], F32, tag="ob", name=f"ob_{i}")
            for c in range(2):
                ps_o = psA.tile([128, 512], F32, tag="ps", name=f"pso_{i}_{c}")
                for k in range(4):
                    nc.tensor.matmul(
                        ps_o, lhsT=hT[:, i, k, :], rhs=wo_sb[:, k, ts(c, 512)],
                        start=(k == 0), stop=(k == 3),
                    )
                nc.vector.tensor_add(
                    out=osb[:, ts(c, 512)], in0=ps_o, in1=xr_t[:, ts(c, 512)]
                )
            nc.sync.dma_start(out=out_ap[ts(i, 128), :], in_=osb)

        # ---- phase A: LN + transpose-DMA; V and pair-0 K/Q inline ----
        x_groups = {}

        def issue_xg(g):
            if g < ST // 4 and g not in x_groups:
                x_groups[g] = xin.tile([128, 4, D], BF, tag="x", name=f"xg_{g}")
                src_ap = bass.AP(
                    tensor=x_ap.tensor,
                    offset=x_ap.offset + g * 512 * D,
                    ap=[[D, 128], [128 * D, 4], [1, D]],
                )
                nc.sync.dma_start(out=x_groups[g], in_=src_ap)

        for g in range(3):
            issue_xg(g)
        for i in range(ST):
            if i == 4:
                issue_xg(3)
            x_t = x_groups[i // 4][:, i % 4, :]
            stats = stat.tile([128, 2, 6], F32, tag="st")
            for sg in range(2):
                nc.vector.bn_stats(out=stats[:, sg, :], in_=x_t[:, ts(sg, 512)])
            mv = stat.tile([128, 2], F32, tag="mv")
            nc.vector.bn_aggr(out=mv, in_=stats)
            std = stat.tile([128, 1], F32, tag="sd")
            nc.scalar.activation(
                out=std, in_=mv[:, 1:2],
                func=mybir.ActivationFunctionType.Sqrt, bias=eps_t,
            )
            istd = stat.tile([128, 1], F32, tag="is")
            nc.vector.reciprocal(out=istd, in_=std)
            xn_t = xnp.tile([128, D], BF, tag="xn")
            nc.vector.tensor_scalar(
                out=xn_t, in0=x_t,
                scalar1=mv[:, 0:1], scalar2=istd,
                op0=mybir.AluOpType.subtract, op1=mybir.AluOpType.mult,
            )
            nc.sync.dma_start_transpose(xnT[:, i], xn_t)
            flush_drains(keep=1)
            if 1 <= i <= 12:
                emit_v(i - 1)
            if i % 4 == 0 and i > 0:
                emit_qk("k", 0, i // 4 - 1, defer=True)
                emit_qk("k", 1, i // 4 - 1, defer=True)
        emit_qk("k", 0, NB - 1, defer=True)
        emit_qk("k", 1, NB - 1, defer=True)
        emit_qk("q", 0, 0, defer=True)
        flush_drains()
        flush_v()
        psV_cm.__exit__(None, None, None)
        psS = ctx.enter_context(tc.tile_pool(name="psS", bufs=2, space="PSUM"))
        psB = ctx.enter_context(tc.tile_pool(name="psB", bufs=1, space="PSUM"))

        # remaining projections stream into the Act-bound attention window,
        # ordered by deadline under the blocks-outer schedule: block (n, p)
        # needs K(p, all) and Q(p, n) before it starts.
        work_queue = (
            [("q", 1, 0)]
            + [("k", 2, n) for n in range(NB)] + [("q", 2, 0)]
            + [("k", 3, 0), ("q", 3, 0), ("k", 3, 1), ("k", 3, 2), ("k", 3, 3)]
            + [("q", p, n) for n in range(1, NB) for p in range(NP_)]
        )

        def pop_work():
            if work_queue:
                emit_qk(*work_queue.pop(0))

        if TSPAN == 2:
            pop_slots = (1, 4, 6)
            out_slots = (3, 7)
        else:
            pop_slots = (3, 8, 13)
            out_slots = (6, 15)

        # ---- phase B: attention, blocks outer so each s-block's output
        # completes early and the out-projection streams across the run ----
        blocks = [(p, n) for n in range(NB) for p in range(NP_)]
        ready_out = []
        s12_next = emit_scores(0, 0, 0)
        for bi, (p, n) in enumerate(blocks):
            psv = [
                psB.tile([128, 4, E + 1], F32, tag=f"pv{h}", name=f"pv_{p}_{n}_{h}")
                for h in range(2)
            ]
            for g in range(GT):
                s12 = s12_next
                pt = ptp.tile([128, TSPAN, 2, 512], BF, tag="pt",
                              name=f"pt_{p}_{n}_{g}")
                nc.scalar.activation(
                    out=pt, in_=s12, func=mybir.ActivationFunctionType.Exp
                )
                # prefetch next scores tile so PE stays busy during exp
                if g < GT - 1:
                    s12_next = emit_scores(p, n, g + 1)
                elif bi < len(blocks) - 1:
                    s12_next = emit_scores(*blocks[bi + 1], 0)
                if bi == 0 and g in (1, 3, 5, 7):
                    emit_v(12 + (g - 1) // 2, streamed=True)
                if g in pop_slots:
                    pop_work()
                if g in out_slots and ready_out:
                    emit_out_tile(ready_out.pop(0))
                for tt in range(TSPAN):
                    t = TSPAN * g + tt
                    for h in range(2):
                        hg = 2 * p + h
                        for sc in range(4):
                            # one start/stop per PSUM bank: the sc==0 start
                            # zeroes the whole 2KB region for all 4 slots
                            nc.tensor.matmul(
                                psv[h][:, sc, :],
                                lhsT=pt[:, tt, h, ts(sc, 128)],
                                rhs=v_sb[:, t, hg, :],
                                start=(t == 0 and sc == 0),
                                stop=(t == TT - 1 and sc == 3),
                                skip_group_check=True,
                            )
            # block epilogue: normalize into hC (denominator is psv[..., E])
            r8 = r8p.tile([128, HL], F32, tag="r8", name=f"r8_{p}_{n}")
            for h in range(2):
                nc.vector.reciprocal(out=r8[:, 4 * h : 4 * h + 4], in_=psv[h][:, :, E])
            for j in range(HL):
                h, sc = j // 4, j % 4
                nc.vector.tensor_scalar_mul(
                    out=hC[:, n * 4 + sc, ts(2 * p + h, E)],
                    in0=psv[h][:, sc, 0:E],
                    scalar1=r8[:, j : j + 1],
                )
            if p == NP_ - 1:
                for sc in range(4):
                    i = n * 4 + sc
                    nc.sync.dma_start_transpose(hT[:, i], hC[:, i, :])
                ready_out.extend(range(n * 4, n * 4 + 4))
        # remaining out-projection tiles
        while ready_out:
            emit_out_tile(ready_out.pop(0))

        aps["_dbg"] = {
            "xnT": xnT.tensor.name, "qT": qT.tensor.name, "kT_": kT_.tensor.name,
            "v_sb": v_sb.tensor.name, "hC": hC.tensor.name, "hT": hT.tensor.name,
        }


def build():
    nc = bacc.Bacc("TRN2", target_bir_lowering=False, debug=False, num_devices=N_CORES)
    aps = {
        "x": nc.dram_tensor("x", [S, D], BF, kind="ExternalInput").ap(),
        "xr": nc.dram_tensor("xr", [S, D], F32, kind="ExternalInput").ap(),
        "wq": nc.dram_tensor("wq", [KT, 128, 512], BF, kind="ExternalInput").ap(),
        "wk": nc.dram_tensor("wk", [KT, 128, 512], BF, kind="ExternalInput").ap(),
        "wv": nc.dram_tensor("wv", [KT, 128, 512], BF, kind="ExternalInput").ap(),
        "wo": nc.dram_tensor("wo", [4, 128, 1024], BF, kind="ExternalInput").ap(),
        "bq": nc.dram_tensor("bq", [128, NP_], F32, kind="ExternalInput").ap(),
        "bk": nc.dram_tensor("bk", [128, NP_], F32, kind="ExternalInput").ap(),
        "out": nc.dram_tensor("out", [S, D], F32, kind="ExternalOutput").ap(),
    }
    with tile.TileContext(nc) as tc:
        aps["tc"] = tc
        _emit(nc, aps)
    nc._dbg_names = aps.get("_dbg", {})
    nc.compile()
    return nc


def prep_core_inputs(x, Wq, bq, Wk, bk, Wv, bv, Wo, bo, ln_gamma, ln_beta):
    """Host-side sharding: returns list of 8 in_maps (numpy arrays)."""
    x = np.asarray(x, np.float32)
    Wq, bq = np.asarray(Wq, np.float32), np.asarray(bq, np.float32)
    Wk, bk = np.asarray(Wk, np.float32), np.asarray(bk, np.float32)
    Wv, bv = np.asarray(Wv, np.float32), np.asarray(bv, np.float32)
    Wo, bo = np.asarray(Wo, np.float32), np.asarray(bo, np.float32)
    gamma, beta = np.asarray(ln_gamma, np.float32), np.asarray(ln_beta, np.float32)

    Wq_eff = Wq * gamma[None, None, :] / SCALE
    bq_eff = (bq + Wq @ beta) / SCALE
    Wk_eff = Wk * gamma[None, None, :]
    bk_eff = bk + Wk @ beta
    Wv_eff = Wv * gamma[None, None, :]
    bv_eff = bv + Wv @ beta

    def wq_layout(w):  # [8, 64, 1024] -> [KT, 128, 512]
        # w[h, e, kt*128+dd] -> out[kt, dd, h*64+e]
        return np.ascontiguousarray(
            w.reshape(HL * E, KT, 128).transpose(1, 2, 0)
        ).astype(BF_NP)

    def b_layout(b):  # [8, 64] -> [128, 4]: out[(hh*64+e), p] = b[2p+hh, e]
        return np.ascontiguousarray(
            b.reshape(NP_, 2 * E).T
        ).astype(np.float32)

    in_maps = []
    for c in range(N_CORES):
        bidx, g = c // 2, c % 2
        hs = slice(g * HL, (g + 1) * HL)
        wo_loc = Wo[:, g * 512:(g + 1) * 512]  # [1024, 512]
        wo_dev = np.ascontiguousarray(
            wo_loc.T.reshape(4, 128, 1024)
        ).astype(BF_NP)
        bv_l = bv_eff[hs].reshape(512)
        in_maps.append({
            "x": x[bidx].astype(BF_NP),
            "xr": 0.5 * (x[bidx] + bo[None, :]) + (wo_loc.astype(np.float32) @ bv_l)[None, :],
            "wq": wq_layout(Wq_eff[hs]),
            "wk": wq_layout(Wk_eff[hs]),
            "wv": wq_layout(Wv_eff[hs]),
            "wo": wo_dev,
            "bq": b_layout(bq_eff[hs]),
            "bk": b_layout(bk_eff[hs]),
            "out": np.zeros((S, D), np.float32),
        })
    return in_maps


def kernel(x, Wq, bq, Wk, bk, Wv, bv, Wo, bo, ln_gamma, ln_beta):
    global _NC_CACHE
    if _NC_CACHE is None:
        _NC_CACHE = build()
    nc = _NC_CACHE
    in_maps = prep_core_inputs(x, Wq, bq, Wk, bk, Wv, bv, Wo, bo, ln_gamma, ln_beta)
    for m in in_maps:
        m.pop("out")
    res = bass_utils.run_bass_kernel_spmd(nc, in_maps, core_ids=list(range(N_CORES)))
    out = np.empty((B, S, D), np.float32)
    for bidx in range(B):
        out[bidx] = res.results[2 * bidx]["out"] + res.results[2 * bidx + 1]["out"]
    return out


# revision 27
# speedup vs baseline: 1.1689x; 1.0012x over previous
"""Multi-head attention block (pre-LN, residual) on 8 Trainium2 NeuronCores.

Sharding: (batch x head-group) grid. Core c handles batch b = c//2 and head
group g = c%2 (8 of 16 heads). Per core: LN(x_b) -> per-head QKV projections
-> softmax attention (no max-subtraction; scores are O(10)) -> out-projection
against the local 512-wide slice of Wo, + 0.5*(x+bo) residual. Host sums the
two partial outputs per batch (the pair all-reduce) and stacks batches.

v2 structure:
- LN apply moved to the Act engine (Identity activation with per-partition
  scale/bias), stats stay on DVE.
- xn -> xnT via DMA crossbar transposes (one per s-tile) instead of PE
  identity-matmul transposes + vector copies.
- PV matmuls flipped to [s, e] output orientation (free size 65 instead of
  512) with a ones-column appended to V so the softmax denominator falls out
  of the PE accumulation for free; normalization is folded into the
  PSUM->SBUF copy. Removes all denominator adds and broadcast-back matmuls.
- Attention heads are concatenated in [s, he] layout (hC) and DMA-crossbar
  transposed per s-tile into hT for the out-projection.
- Scores accumulate in bf16 PSUM so an exp instruction spans 2 t-tiles
  (2048 elements), halving Act-engine instruction overhead.

LayerNorm gamma/beta are folded into the QKV weights/biases on the host
(exact: projections are linear in xn). The 1/sqrt(E) score scale is folded
into Wq. Matmul operands are bf16 with fp32 PSUM accumulation for
projections/PV/out-proj; LN statistics and the residual path stay fp32.
"""

import numpy as np
import ml_dtypes

import concourse.bass as bass
import concourse.mybir as mybir
import concourse.tile as tile
from concourse import bacc
from concourse import bass_utils
from concourse.bass import ts

BF_NP = ml_dtypes.bfloat16

B, S, D = 4, 2048, 1024
H, E = 16, 64
LN_EPS = 1e-5
SCALE = 8.0  # sqrt(E) * TEMP

N_CORES = 8
HL = H // 2          # heads per core
NP_ = HL // 2        # 4 head pairs per core
KT = D // 128        # 8 contraction tiles for D
ST = S // 128        # 16 s-tiles of 128
NB = S // 512        # 4 s-blocks of 512
TT = S // 128        # 16 t-tiles of 128

F32 = mybir.dt.float32
BF = mybir.dt.bfloat16

# Score PSUM dtype: bf16 lets one exp instruction span TSPAN=2 t-tiles within
# the PSUM budget. Fallback (if bf16 matmul-out to PSUM misbehaves):
# F32S=True -> f32 scores, TSPAN=1.
F32S = True
TSPAN = 1 if F32S else 2
GT = TT // TSPAN     # groups per block
SDT = F32 if F32S else BF

_NC_CACHE = None


def _emit(nc, aps):
    x_ap = aps["x"]
    xr_ap = aps["xr"]
    wq_ap, wk_ap, wv_ap, wo_ap = aps["wq"], aps["wk"], aps["wv"], aps["wo"]
    bq_ap, bk_ap = aps["bq"], aps["bk"]
    out_ap = aps["out"]

    tc = aps["tc"]
    import contextlib

    ctx = contextlib.ExitStack()
    with ctx:
        const = ctx.enter_context(tc.tile_pool(name="const", bufs=1))
        big = ctx.enter_context(tc.tile_pool(name="big", bufs=1))
        xin = ctx.enter_context(tc.tile_pool(name="xin", bufs=3))
        stat = ctx.enter_context(tc.tile_pool(name="stat", bufs=8))
        xnp = ctx.enter_context(tc.tile_pool(name="xnp", bufs=6))
        ptp = ctx.enter_context(tc.tile_pool(name="ptp", bufs=4))
        r8p = ctx.enter_context(tc.tile_pool(name="r8p", bufs=2))
        xrp = ctx.enter_context(tc.tile_pool(name="xrp", bufs=2))
        outp = ctx.enter_context(tc.tile_pool(name="outp", bufs=2))
        psA = ctx.enter_context(tc.tile_pool(name="psA", bufs=2, space="PSUM"))
        # psV is phase-A scoped: its 4 banks are released before the
        # attention pools (psS/psB) are created, so V-projection drains can
        # lag 3 tiles behind their matmuls without blocking the ring.
        psV_cm = tc.tile_pool(name="psV", bufs=4, space="PSUM")
        psV = psV_cm.__enter__()

        # ---- constants / weights resident in SBUF ----
        # wv/wk ride the fast HWDGE (sync) queue right after the first x
        # tiles: V(0) needs wv at ~7us, K(0,0) needs wk at ~10us. The SWDGE
        # (gpsimd) path costs ~1us Pool-engine time per DMA, which starved
        # the projections when all weights went through it.
        wq_sb = const.tile([128, KT, 512], BF, tag="wq")
        wk_sb = const.tile([128, KT, 512], BF, tag="wk")
        wv_sb = const.tile([128, KT, 512], BF, tag="wv")
        wo_sb = const.tile([128, 4, 1024], BF, tag="wo")
        bq_sb = const.tile([128, NP_], F32, tag="bq")
        bk_sb = const.tile([128, NP_], F32, tag="bk")
        # all weights ride the gpsimd SWDGE queue (Pool engine is otherwise
        # idle), most-urgent first: wv for V(0) at ~9us, wk for K(p0),
        # wq/wo much later. Keeps both HWDGE queues free for x tiles and
        # transposes. bv is folded into xr on the host (exact: the softmax
        # rows sum to one, so +bv passes through attention untouched).
        nc.gpsimd.dma_start(out=bq_sb, in_=bq_ap)
        nc.gpsimd.dma_start(out=bk_sb, in_=bk_ap)

        def w_ap(src, kt, free):
            return bass.AP(tensor=src.tensor, offset=src.offset,
                           ap=[[free, 128], [128 * free, kt], [1, free]])

        nc.gpsimd.dma_start(out=wv_sb, in_=w_ap(wv_ap, KT, 512))
        nc.gpsimd.dma_start(out=wk_sb, in_=w_ap(wk_ap, KT, 512))
        nc.gpsimd.dma_start(out=wq_sb, in_=w_ap(wq_ap, KT, 512))
        nc.gpsimd.dma_start(out=wo_sb, in_=w_ap(wo_ap, 4, 1024))
        eps_t = const.tile([128, 1], F32, tag="eps")
        nc.vector.memset(eps_t, LN_EPS)

        # [d%128, s_tile, d//128, s%128] LN(x) transposed, per-tile contiguous
        xnT = big.tile([128, ST, KT, 128], BF, tag="xnT")
        qT = big.tile([128, NP_, S], BF, tag="qT")      # [(pairhead,e), s]
        kT_ = big.tile([128, NP_, S], BF, tag="kT")
        v_sb = big.tile([128, TT, HL, E + 1], BF, tag="v")  # [t, h, e | ones]
        hC = big.tile([128, ST, 512], BF, tag="hC")     # [s, (h,e)] attn out
        hT = big.tile([128, ST, 4, 128], BF, tag="hT")  # [(he)%128, s_tile, (he)//128, s%128]
        # ones column of V' (denominator accumulator feed)
        nc.gpsimd.memset(v_sb[:, :, :, E : E + 1], 1.0)

        # ---- work units ----
        # Phase-A PSUM drains are deferred to the next tile so no in-order
        # DVE instruction ever waits on the transpose->matmul chain (a DVE
        # stall there blocks the next tile's LN stats behind it).
        drains = []

        def flush_drains(keep=0):
            while len(drains) > keep:
                drains.pop(0)()

        def emit_qk(kind, p, n, defer=False):
            # never leave >1 undrained psA unit: the ring has 2 slots and a
            # writer emitted before the previous reader is a race
            flush_drains(keep=1)
            w_sb, b_sb, dst = (
                (wq_sb, bq_sb, qT) if kind == "q" else (wk_sb, bk_sb, kT_)
            )
            ps = psA.tile([128, 512], F32, tag="ps", name=f"proj_{kind}_{p}_{n}")
            for k in range(KT):
                nc.tensor.matmul(
                    ps, lhsT=w_sb[:, k, ts(p, 128)],
                    rhs=xnT[:, 4 * n : 4 * n + 4, k, :],
                    start=(k == 0), stop=(k == KT - 1),
                )

            def drain_act():
                # per-partition bias add on the Act engine (phase A: DVE is
                # saturated by the LN chain, Act is idle)
                nc.scalar.activation(
                    out=dst[:, p, ts(n, 512)], in_=ps,
                    func=mybir.ActivationFunctionType.Identity,
                    bias=b_sb[:, p : p + 1],
                )

            def drain_dve():
                nc.vector.tensor_scalar_add(
                    out=dst[:, p, ts(n, 512)], in0=ps, scalar1=b_sb[:, p : p + 1]
                )

            if defer:
                drains.append(drain_act)
            else:
                drain_dve()

        v_drains = []

        def flush_v(keep=0):
            while len(v_drains) > keep:
                v_drains.pop(0)()

        def emit_v(t, streamed=False):
            if streamed:
                ps = psA.tile([128, HL, E], F32, tag="ps", name=f"proj_v_{t}")
            else:
                flush_v(keep=3)
                ps = psV.tile([128, HL, E], F32, tag="psv", name=f"proj_v_{t}")
            for k in range(KT):
                nc.tensor.matmul(
                    ps, lhsT=xnT[:, t, k, :], rhs=wv_sb[:, k, :],
                    start=(k == 0), stop=(k == KT - 1),
                )
            if streamed:
                nc.vector.tensor_copy(out=v_sb[:, t, :, 0:E], in_=ps)
            else:
                v_drains.append(
                    lambda: nc.scalar.copy(out=v_sb[:, t, :, 0:E], in_=ps)
                )

        def emit_scores(p, n, g):
            s12 = psS.tile([128, TSPAN, 2, 512], SDT, tag="s12",
                           name=f"s12_{p}_{n}_{g}")
            for tt in range(TSPAN):
                t = TSPAN * g + tt
                nc.tensor.matmul(
                    s12[:, tt, 0, :], lhsT=kT_[0:64, p, ts(t, 128)],
                    rhs=qT[0:64, p, ts(n, 512)],
                    start=True, stop=True, tile_position=(0, 0),
                )
                nc.tensor.matmul(
                    s12[:, tt, 1, :], lhsT=kT_[64:128, p, ts(t, 128)],
                    rhs=qT[64:128, p, ts(n, 512)],
                    start=True, stop=True, tile_position=(64, 0),
                )
            return s12

        def emit_out_tile(i):
            xr_t = xrp.tile([128, D], F32, tag="xr", name=f"xr_{i}")
            nc.gpsimd.dma_start(out=xr_t, in_=xr_ap[ts(i, 128), :])
            osb = outp.tile([128, D], F32, tag="ob", name=f"ob_{i}")
            for c in range(2):
                ps_o = psA.tile([128, 512], F32, tag="ps", name=f"pso_{i}_{c}")
                for k in range(4):
                    nc.tensor.matmul(
                        ps_o, lhsT=hT[:, i, k, :], rhs=wo_sb[:, k, ts(c, 512)],
                        start=(k == 0), stop=(k == 3),
                    )
                nc.vector.tensor_add(
                    out=osb[:, ts(c, 512)], in0=ps_o, in1=xr_t[:, ts(c, 512)]
                )
            nc.sync.dma_start(out=out_ap[ts(i, 128), :], in_=osb)

        # ---- phase A: LN + transpose-DMA; V and pair-0 K/Q inline ----
        x_groups = {}

        def issue_xg(g):
            if g < ST // 4 and g not in x_groups:
                x_groups[g] = xin.tile([128, 4, D], BF, tag="x", name=f"xg_{g}")
                src_ap = bass.AP(
                    tensor=x_ap.tensor,
                    offset=x_ap.offset + g * 512 * D,
                    ap=[[D, 128], [128 * D, 4], [1, D]],
                )
                nc.sync.dma_start(out=x_groups[g], in_=src_ap)

        for g in range(3):
            issue_xg(g)
        for i in range(ST):
            if i == 4:
                issue_xg(3)
            x_t = x_groups[i // 4][:, i % 4, :]
            stats = stat.tile([128, 2, 6], F32, tag="st")
            for sg in range(2):
                nc.vector.bn_stats(out=stats[:, sg, :], in_=x_t[:, ts(sg, 512)])
            mv = stat.tile([128, 2], F32, tag="mv")
            nc.vector.bn_aggr(out=mv, in_=stats)
            std = stat.tile([128, 1], F32, tag="sd")
            nc.scalar.activation(
                out=std, in_=mv[:, 1:2],
                func=mybir.ActivationFunctionType.Sqrt, bias=eps_t,
            )
            istd = stat.tile([128, 1], F32, tag="is")
            nc.vector.reciprocal(out=istd, in_=std)
            xn_t = xnp.tile([128, D], BF, tag="xn")
            nc.vector.tensor_scalar(
                out=xn_t, in0=x_t,
                scalar1=mv[:, 0:1], scalar2=istd,
                op0=mybir.AluOpType.subtract, op1=mybir.AluOpType.mult,
            )
            nc.sync.dma_start_transpose(xnT[:, i], xn_t)
            flush_drains(keep=1)
            if 1 <= i <= 12:
                emit_v(i - 1)
            if i % 4 == 0 and i > 0:
                emit_qk("k", 0, i // 4 - 1, defer=True)
                emit_qk("k", 1, i // 4 - 1, defer=True)
        emit_qk("k", 0, NB - 1, defer=True)
        emit_qk("k", 1, NB - 1, defer=True)
        emit_qk("q", 0, 0, defer=True)
        flush_drains()
        flush_v()
        psV_cm.__exit__(None, None, None)
        psS = ctx.enter_context(tc.tile_pool(name="psS", bufs=2, space="PSUM"))
        psB = ctx.enter_context(tc.tile_pool(name="psB", bufs=1, space="PSUM"))

        # remaining projections stream into the Act-bound attention window,
        # ordered by deadline under the blocks-outer schedule: block (n, p)
        # needs K(p, all) and Q(p, n) before it starts.
        work_queue = (
            [("q", 1, 0)]
            + [("k", 2, n) for n in range(NB)] + [("q", 2, 0)]
            + [("k", 3, 0), ("q", 3, 0), ("k", 3, 1), ("k", 3, 2), ("k", 3, 3)]
            + [("q", p, n) for n in range(1, NB) for p in range(NP_)]
        )

        def pop_work():
            if work_queue:
                emit_qk(*work_queue.pop(0))

        if TSPAN == 2:
            pop_slots = (1, 4, 6)
            out_slots = (3, 7)
        else:
            pop_slots = (3, 8, 13)
            out_slots = (6, 15)

        # ---- phase B: attention, blocks outer so each s-block's output
        # completes early and the out-projection streams across the run ----
        blocks = [(p, n) for n in range(NB) for p in range(NP_)]
        ready_out = []
        s12_next = emit_scores(0, 0, 0)
        for bi, (p, n) in enumerate(blocks):
            psv = [
                psB.tile([128, 4, E + 1], F32, tag=f"pv{h}", name=f"pv_{p}_{n}_{h}")
                for h in range(2)
            ]
            for g in range(GT):
                s12 = s12_next
                pt = ptp.tile([128, TSPAN, 2, 512], BF, tag="pt",
                              name=f"pt_{p}_{n}_{g}")
                nc.scalar.activation(
                    out=pt, in_=s12, func=mybir.ActivationFunctionType.Exp
                )
                # prefetch next scores tile so PE stays busy during exp
                if g < GT - 1:
                    s12_next = emit_scores(p, n, g + 1)
                elif bi < len(blocks) - 1:
                    s12_next = emit_scores(*blocks[bi + 1], 0)
                if bi == 0 and g in (1, 3, 5, 7):
                    emit_v(12 + (g - 1) // 2, streamed=True)
                if g in pop_slots:
                    pop_work()
                if g in out_slots and ready_out:
                    emit_out_tile(ready_out.pop(0))
                for tt in range(TSPAN):
                    t = TSPAN * g + tt
                    for h in range(2):
                        hg = 2 * p + h
                        for sc in range(4):
                            # one start/stop per PSUM bank: the sc==0 start
                            # zeroes the whole 2KB region for all 4 slots
                            nc.tensor.matmul(
                                psv[h][:, sc, :],
                                lhsT=pt[:, tt, h, ts(sc, 128)],
                                rhs=v_sb[:, t, hg, :],
                                start=(t == 0 and sc == 0),
                                stop=(t == TT - 1 and sc == 3),
                                skip_group_check=True,
                            )
            # block epilogue: normalize into hC (denominator is psv[..., E])
            r8 = r8p.tile([128, HL], F32, tag="r8", name=f"r8_{p}_{n}")
            for h in range(2):
                nc.vector.reciprocal(out=r8[:, 4 * h : 4 * h + 4], in_=psv[h][:, :, E])
            for j in range(HL):
                h, sc = j // 4, j % 4
                nc.vector.tensor_scalar_mul(
                    out=hC[:, n * 4 + sc, ts(2 * p + h, E)],
                    in0=psv[h][:, sc, 0:E],
                    scalar1=r8[:, j : j + 1],
                )
            if p == NP_ - 1:
                for sc in range(4):
                    i = n * 4 + sc
                    nc.sync.dma_start_transpose(hT[:, i], hC[:, i, :])
                ready_out.extend(range(n * 4, n * 4 + 4))
        # remaining out-projection tiles
        while ready_out:
            emit_out_tile(ready_out.pop(0))

        aps["_dbg"] = {
            "xnT": xnT.tensor.name, "qT": qT.tensor.name, "kT_": kT_.tensor.name,
            "v_sb": v_sb.tensor.name, "hC": hC.tensor.name, "hT": hT.tensor.name,
        }


def build():
    nc = bacc.Bacc("TRN2", target_bir_lowering=False, debug=False, num_devices=N_CORES)
    aps = {
        "x": nc.dram_tensor("x", [S, D], BF, kind="ExternalInput").ap(),
        "xr": nc.dram_tensor("xr", [S, D], F32, kind="ExternalInput").ap(),
        "wq": nc.dram_tensor("wq", [KT, 128, 512], BF, kind="ExternalInput").ap(),
        "wk": nc.dram_tensor("wk", [KT, 128, 512], BF, kind="ExternalInput").ap(),
        "wv": nc.dram_tensor("wv", [KT, 128, 512], BF, kind="ExternalInput").ap(),
        "wo": nc.dram_tensor("wo", [4, 128, 1024], BF, kind="ExternalInput").ap(),
        "bq": nc.dram_tensor("bq", [128, NP_], F32, kind="ExternalInput").ap(),
        "bk": nc.dram_tensor("bk", [128, NP_], F32, kind="ExternalInput").ap(),
        "out": nc.dram_tensor("out", [S, D], F32, kind="ExternalOutput").ap(),
    }
    with tile.TileContext(nc) as tc:
        aps["tc"] = tc
        _emit(nc, aps)
    nc._dbg_names = aps.get("_dbg", {})
    nc.compile()
    return nc


def prep_core_inputs(x, Wq, bq, Wk, bk, Wv, bv, Wo, bo, ln_gamma, ln_beta):
    """Host-side sharding: returns list of 8 in_maps (numpy arrays)."""
    x = np.asarray(x, np.float32)
    Wq, bq = np.asarray(Wq, np.float32), np.asarray(bq, np.float32)
    Wk, bk = np.asarray(Wk, np.float32), np.asarray(bk, np.float32)
    Wv, bv = np.asarray(Wv, np.float32), np.asarray(bv, np.float32)
    Wo, bo = np.asarray(Wo, np.float32), np.asarray(bo, np.float32)
    gamma, beta = np.asarray(ln_gamma, np.float32), np.asarray(ln_beta, np.float32)

    Wq_eff = Wq * gamma[None, None, :] / SCALE
    bq_eff = (bq + Wq @ beta) / SCALE
    Wk_eff = Wk * gamma[None, None, :]
    bk_eff = bk + Wk @ beta
    Wv_eff = Wv * gamma[None, None, :]
    bv_eff = bv + Wv @ beta

    def wq_layout(w):  # [8, 64, 1024] -> [KT, 128, 512]
        # w[h, e, kt*128+dd] -> out[kt, dd, h*64+e]
        return np.ascontiguousarray(
            w.reshape(HL * E, KT, 128).transpose(1, 2, 0)
        ).astype(BF_NP)

    def b_layout(b):  # [8, 64] -> [128, 4]: out[(hh*64+e), p] = b[2p+hh, e]
        return np.ascontiguousarray(
            b.reshape(NP_, 2 * E).T
        ).astype(np.float32)

    in_maps = []
    for c in range(N_CORES):
        bidx, g = c // 2, c % 2
        hs = slice(g * HL, (g + 1) * HL)
        wo_loc = Wo[:, g * 512:(g + 1) * 512]  # [1024, 512]
        wo_dev = np.ascontiguousarray(
            wo_loc.T.reshape(4, 128, 1024)
        ).astype(BF_NP)
        bv_l = bv_eff[hs].reshape(512)
        in_maps.append({
            "x": x[bidx].astype(BF_NP),
            "xr": 0.5 * (x[bidx] + bo[None, :]) + (wo_loc.astype(np.float32) @ bv_l)[None, :],
            "wq": wq_layout(Wq_eff[hs]),
            "wk": wq_layout(Wk_eff[hs]),
            "wv": wq_layout(Wv_eff[hs]),
            "wo": wo_dev,
            "bq": b_layout(bq_eff[hs]),
            "bk": b_layout(bk_eff[hs]),
            "out": np.zeros((S, D), np.float32),
        })
    return in_maps


def kernel(x, Wq, bq, Wk, bk, Wv, bv, Wo, bo, ln_gamma, ln_beta):
    global _NC_CACHE
    if _NC_CACHE is None:
        _NC_CACHE = build()
    nc = _NC_CACHE
    in_maps = prep_core_inputs(x, Wq, bq, Wk, bk, Wv, bv, Wo, bo, ln_gamma, ln_beta)
    for m in in_maps:
        m.pop("out")
    res = bass_utils.run_bass_kernel_spmd(nc, in_maps, core_ids=list(range(N_CORES)))
    out = np.empty((B, S, D), np.float32)
    for bidx in range(B):
        out[bidx] = res.results[2 * bidx]["out"] + res.results[2 * bidx + 1]["out"]
    return out
